# revision 1
# baseline (speedup 1.0000x reference)
# Trainium2 Bass kernel for nn_DirectedMessagePassing (chemprop-style DMPNN).
#
# Device executes all dense compute (the per-edge MLPs and readout MLP) as
# Bass/Tile kernels SPMD across the 8 NeuronCores, with edges/nodes sharded
# 1/8 per core.  The index-only segment-sum / gather plumbing between the
# dense stages runs on the host (this deployment's runtime cannot execute
# the SWDGE gather/scatter-add or DynamicAP DMA instructions — verified by
# direct experiment — so data-dependent addressing cannot run on-device).
#
#   h0 = relu(ea @ W0 + b0)                          (device, edge-sharded)
#   per step: ns = segment_sum(h by tgt); t = ns @ W1b + b1 (host, index ops)
#             agg1[e] = t[src[e]] - rev corrections   (host gather)
#             m = relu(relu([h;agg1] @ [W1t;I]) @ W2 + b2)  (device)
#             h += m                                  (host add)
#   out = relu([x, segsum(h)] @ W3 + b3) @ W4 + b4    (device, node-sharded)
import numpy as np

import concourse.bacc as bacc
import concourse.mybir as mybir
import concourse.tile as tile
from concourse.bass_utils import run_bass_kernel_spmd

N_NODES = 40000
N_EDGES = 400000
EDGE_F = 16
HID = 64
STEPS = 3
NCORES = 8

F32 = mybir.dt.float32
AF = mybir.ActivationFunctionType

EC = 50176          # padded edges per core (98 chunks of 512)
NC_SH = 5120        # padded nodes per core (10 chunks of 512)


def _build_edge_mlp(first):
    """first=True: in = ea_aug [17, EC]   -> h0_em [EC, 64]
       first=False: in = z [128, EC] (h;agg1) -> m_em [EC, 64]"""
    nc = bacc.Bacc(trn_type="TRN2", num_devices=NCORES)
    KIN = EDGE_F + 1 if first else 128
    zin = nc.dram_tensor("zin", [KIN, EC], F32, kind="ExternalInput")
    wA = nc.dram_tensor("wA", [KIN, HID], F32, kind="ExternalInput")
    wB = nc.dram_tensor("wB", [HID + 1, HID], F32, kind="ExternalInput")
    out = nc.dram_tensor("out", [EC, HID], F32, kind="ExternalOutput")
    with tile.TileContext(nc) as tc:
        with (
            tc.tile_pool(name="const", bufs=1) as constp,
            tc.tile_pool(name="work", bufs=3) as workp,
            tc.tile_pool(name="ps1", bufs=2, space="PSUM") as ps1p,
            tc.tile_pool(name="ps2", bufs=2, space="PSUM") as ps2p,
        ):
            wAt = constp.tile([KIN, HID], F32, name="wAt")
            nc.sync.dma_start(out=wAt[:], in_=wA[:])
            wBt = constp.tile([HID + 1, HID], F32, name="wBt")
            nc.sync.dma_start(out=wBt[:], in_=wB[:])
            for ch in range(EC // 512):
                zt = workp.tile([KIN, 512], F32, name="zt", tag="zt")
                nc.sync.dma_start(out=zt[:], in_=zin[:, ch * 512:(ch + 1) * 512])
                if first:
                    pse = ps2p.tile([128, 256], F32, name="pse", tag="ps2")
                    for b in range(4):
                        nc.tensor.matmul(pse[:, b * HID:(b + 1) * HID],
                                         zt[:, b * 128:(b + 1) * 128], wAt[:],
                                         start=True, stop=True)
                    ot = workp.tile([128, 4, HID], F32, name="ot", tag="ot")
                    nc.scalar.activation(
                        ot[:], pse[:].rearrange("p (b h) -> p b h", b=4), AF.Relu)
                else:
                    ps1 = ps1p.tile([64, 512], F32, name="ps1", tag="ps1")
                    nc.tensor.matmul(ps1[:], wAt[:], zt[:], start=True, stop=True)
                    m1 = workp.tile([HID + 1, 512], F32, name="m1", tag="m1")
                    nc.vector.memset(m1[HID:HID + 1, :], 1.0)
                    nc.scalar.activation(m1[0:HID, :], ps1[:], AF.Relu)
                    pse = ps2p.tile([128, 256], F32, name="pse2", tag="ps2")
                    for b in range(4):
                        nc.tensor.matmul(pse[:, b * HID:(b + 1) * HID],
                                         m1[:, b * 128:(b + 1) * 128], wBt[:],
                                         start=True, stop=True)
                    ot = workp.tile([128, 4, HID], F32, name="ot2", tag="ot")
                    nc.scalar.activation(
                        ot[:], pse[:].rearrange("p (b h) -> p b h", b=4), AF.Relu)
                nc.scalar.dma_start(
                    out=out[ch * 512:(ch + 1) * 512, :].rearrange(
                        "(b p) h -> p b h", p=128),
                    in_=ot[:])
    nc.finalize()
    return nc


def _build_readout():
    """in: x_fm [128, NC_SH], amo [65, NC_SH] (atom_msg; ones) -> out [NC_SH, 64]"""
    nc = bacc.Bacc(trn_type="TRN2", num_devices=NCORES)
    x_fm = nc.dram_tensor("x_fm", [128, NC_SH], F32, kind="ExternalInput")
    amo = nc.dram_tensor("amo", [HID + 1, NC_SH], F32, kind="ExternalInput")
    w3x = nc.dram_tensor("w3x", [128, HID], F32, kind="ExternalInput")
    w3m = nc.dram_tensor("w3m", [HID + 1, HID], F32, kind="ExternalInput")
    w4a = nc.dram_tensor("w4a", [HID + 1, HID], F32, kind="ExternalInput")
    out = nc.dram_tensor("out", [NC_SH, HID], F32, kind="ExternalOutput")
    with tile.TileContext(nc) as tc:
        with (
            tc.tile_pool(name="const", bufs=1) as constp,
            tc.tile_pool(name="work", bufs=3) as workp,
            tc.tile_pool(name="ps1", bufs=2, space="PSUM") as ps1p,
            tc.tile_pool(name="ps2", bufs=2, space="PSUM") as ps2p,
        ):
            w3xt = constp.tile([128, HID], F32, name="w3xt")
            nc.sync.dma_start(out=w3xt[:], in_=w3x[:])
            w3mt = constp.tile([HID + 1, HID], F32, name="w3mt")
            nc.sync.dma_start(out=w3mt[:], in_=w3m[:])
            w4at = constp.tile([HID + 1, HID], F32, name="w4at")
            nc.sync.dma_start(out=w4at[:], in_=w4a[:])
            for ch in range(NC_SH // 512):
                xt = workp.tile([128, 512], F32, name="xt", tag="xt")
                nc.sync.dma_start(out=xt[:], in_=x_fm[:, ch * 512:(ch + 1) * 512])
                at = workp.tile([HID + 1, 512], F32, name="at", tag="at")
                nc.sync.dma_start(out=at[:], in_=amo[:, ch * 512:(ch + 1) * 512])
                psr = ps1p.tile([64, 512], F32, name="psr", tag="ps1")
                nc.tensor.matmul(psr[:], w3xt[:], xt[:], start=True, stop=False)
                nc.tensor.matmul(psr[:], w3mt[:], at[:], start=False, stop=True)
                r1 = workp.tile([HID + 1, 512], F32, name="r1", tag="r1")
                nc.vector.memset(r1[HID:HID + 1, :], 1.0)
                nc.scalar.activation(r1[0:HID, :], psr[:], AF.Relu)
                pso = ps2p.tile([128, 256], F32, name="pso", tag="ps2")
                for b in range(4):
                    nc.tensor.matmul(pso[:, b * HID:(b + 1) * HID],
                                     r1[:, b * 128:(b + 1) * 128], w4at[:],
                                     start=True, stop=True)
                ot = workp.tile([128, 4, HID], F32, name="rot", tag="rot")
                nc.vector.tensor_copy(ot[:], pso[:].rearrange("p (b h) -> p b h", b=4))
                nc.scalar.dma_start(
                    out=out[ch * 512:(ch + 1) * 512, :].rearrange(
                        "(b p) h -> p b h", p=128),
                    in_=ot[:])
    nc.finalize()
    return nc


_CACHE = {}


def _get(name, builder, *a):
    if name not in _CACHE:
        _CACHE[name] = builder(*a)
    return _CACHE[name]


def _run_edge(nc, zin_full, wA, wB, trace=False):
    in_maps = [{"zin": np.ascontiguousarray(zin_full[:, c * EC:(c + 1) * EC]),
                "wA": wA, "wB": wB} for c in range(NCORES)]
    res = run_bass_kernel_spmd(nc, in_maps, list(range(NCORES)), trace=trace)
    return np.concatenate([res.results[c]["out"] for c in range(NCORES)], axis=0), res


def kernel(**inputs):
    x = np.asarray(inputs["x"], np.float32)
    edge_index = np.asarray(inputs["edge_index"])
    ea = np.asarray(inputs["edge_attr"], np.float32)
    W0, b0, W1, b1, W2, b2, W3, b3, W4, b4 = (
        np.asarray(inputs[k], np.float32) for k in
        ["W0", "b0", "W1", "b1", "W2", "b2", "W3", "b3", "W4", "b4"])
    src = edge_index[0].astype(np.int64)
    tgt = edge_index[1].astype(np.int64)
    E = src.shape[0]
    W1t, W1b = W1[:HID], W1[HID:]

    # reverse-pair structure (exactly the reference's construction)
    key = src * N_NODES + tgt
    order = np.argsort(key, kind="stable")
    key_sorted = key[order]
    rev_key = tgt * N_NODES + src
    lo = np.searchsorted(key_sorted, rev_key, side="left")
    hi = np.searchsorted(key_sorted, rev_key, side="right")
    special = np.nonzero(hi > lo)[0]

    # tgt-sorted order for fast host segment sums
    torder = np.argsort(tgt, kind="stable")
    tsorted = tgt[torder]
    seg_starts = np.searchsorted(tsorted, np.arange(N_NODES), side="left")
    reduce_idx = np.minimum(seg_starts, E - 1)
    empty = seg_starts == np.append(seg_starts[1:], E)

    def segsum(vals):
        s = np.add.reduceat(vals[torder], reduce_idx, axis=0)
        s[empty] = 0.0
        return s

    EPAD = NCORES * EC

    # ---- h0 on device ----
    nc_first = _get("first", _build_edge_mlp, True)
    ea_aug = np.zeros((EDGE_F + 1, EPAD), np.float32)
    ea_aug[EDGE_F] = 1.0
    ea_aug[:EDGE_F, :E] = ea.T
    wA0 = np.concatenate([W0, b0[None, :]], axis=0).astype(np.float32)
    wB_dummy = np.zeros((HID + 1, HID), np.float32)
    h_full, _ = _run_edge(nc_first, ea_aug, wA0, wB_dummy)
    h = h_full[:E]

    # ---- steps ----
    nc_step = _get("step", _build_edge_mlp, False)
    wA1 = np.concatenate([W1t, np.eye(HID, dtype=np.float32)], axis=0)
    wB1 = np.concatenate([W2, b2[None, :]], axis=0).astype(np.float32)
    ns = segsum(h)
    for _ in range(STEPS):
        t = ns @ W1b + b1
        agg1 = t[src]
        for e in special:
            rev = h[order[lo[e]:hi[e]]].sum(axis=0)
            agg1[e] = (ns[src[e]] - rev) @ W1b + b1
        z = np.zeros((128, EPAD), np.float32)
        z[:HID, :E] = h.T
        z[HID:, :E] = agg1.T
        m_full, _ = _run_edge(nc_step, z, wA1, wB1)
        m = m_full[:E]
        h = h + m
        ns = ns + segsum(m)

    # ---- readout on device ----
    nc_ro = _get("readout", _build_readout)
    w3xm = W3[:128].astype(np.float32)
    w3mm = np.concatenate([W3[128:], b3[None, :]], axis=0).astype(np.float32)
    w4am = np.concatenate([W4, b4[None, :]], axis=0).astype(np.float32)
    in_maps = []
    NSH = N_NODES // NCORES
    for c in range(NCORES):
        xf = np.zeros((128, NC_SH), np.float32)
        xf[:, :NSH] = x[c * NSH:(c + 1) * NSH].T
        am = np.zeros((HID + 1, NC_SH), np.float32)
        am[HID] = 1.0
        am[:HID, :NSH] = ns[c * NSH:(c + 1) * NSH].T
        in_maps.append({"x_fm": xf, "amo": am, "w3x": w3xm, "w3m": w3mm,
                        "w4a": w4am})
    res = run_bass_kernel_spmd(nc_ro, in_maps, list(range(NCORES)))
    outs = [res.results[c]["out"][:NSH] for c in range(NCORES)]
    return np.concatenate(outs, axis=0).astype(np.float32)



# revision 7
# speedup vs baseline: 1.6324x; 1.6324x over previous
# Trainium2 Bass kernel for nn_DirectedMessagePassing (chemprop-style DMPNN).
#
# Device executes all dense compute (per-edge MLPs and readout MLP) as
# Bass/Tile kernels SPMD across the 8 NeuronCores, edges/nodes sharded 1/8
# per core.  Index-only segment-sum / gather plumbing between the dense
# stages runs on the host.
#
# All device I/O is fp16 in transposed [feat, elem] layout, batched into
# 4096-elem DMA slabs (8KB/partition; HWDGE fixed overhead ~650ns per DMA
# instruction would otherwise dominate).  Matmuls run in fp16 at the
# 512-col moving-operand max, two matmuls per [64, 1024] PSUM tile so the
# activation / bias ops run at 1024 cols per instruction.  Biases are folded
# into activation/vector ops.
#
#   h0 = relu(ea @ W0 + b0)                          (device, edge-sharded)
#   per step: ns = segment_sum(h by tgt); t = ns @ W1b + b1   (host)
#             agg1[e] = t[src[e]] - rev corrections          (host gather)
#             m = relu(relu(h@W1t + agg1) @ W2 + b2)          (device)
#             h += m                                          (host add)
#   out = relu([x, segsum(h)] @ W3 + b3) @ W4 + b4    (device, node-sharded)
import numpy as np

import concourse.bacc as bacc
import concourse.mybir as mybir
import concourse.tile as tile
from concourse.bass_utils import run_bass_kernel_spmd

N_NODES = 40000
N_EDGES = 400000
EDGE_F = 16
HID = 64
STEPS = 3
NCORES = 8

F32 = mybir.dt.float32
F16 = mybir.dt.float16
AF = mybir.ActivationFunctionType
ALU = mybir.AluOpType

MM = 512            # free-dim cols per matmul (fp16 moving-operand max)
CH = 1024           # cols per PSUM tile / activation instruction
SLAB = 4096         # elems per DMA instruction (8KB/partition in fp16)
EC = 50176          # padded edges per core (12*4096 + 1024)
NC_SH = 5120        # padded nodes per core


def _slabs(total):
    s = 0
    while s < total:
        sz = min(SLAB, total - s)
        yield s, sz
        s += sz


def _build_first():
    """in: ea_t [16, EC] f16 -> h0t [64, EC] f16  (relu(ea@W0+b0), transposed)"""
    nc = bacc.Bacc(trn_type="TRN2", num_devices=NCORES)
    ea_t = nc.dram_tensor("ea_t", [EDGE_F, EC], F16, kind="ExternalInput")
    w0 = nc.dram_tensor("w0", [EDGE_F, HID], F16, kind="ExternalInput")
    b0c = nc.dram_tensor("b0c", [HID, 1], F32, kind="ExternalInput")
    out = nc.dram_tensor("out", [HID, EC], F16, kind="ExternalOutput")
    with tile.TileContext(nc) as tc:
        with (
            tc.tile_pool(name="const", bufs=1) as constp,
            tc.tile_pool(name="work", bufs=3) as workp,
            tc.tile_pool(name="ps", bufs=2, space="PSUM") as psp,
        ):
            w0t = constp.tile([EDGE_F, HID], F16, name="w0t")
            nc.sync.dma_start(out=w0t[:], in_=w0[:])
            b0t = constp.tile([HID, 1], F32, name="b0t")
            nc.scalar.dma_start(out=b0t[:], in_=b0c[:])
            for s0, ssz in _slabs(EC):
                es = workp.tile([EDGE_F, SLAB], F16, name="es", tag="es")
                nc.sync.dma_start(out=es[:, :ssz], in_=ea_t[:, s0:s0 + ssz])
                os = workp.tile([HID, SLAB], F16, name="os", tag="os")
                for k in range(ssz // CH):
                    cs = slice(k * CH, (k + 1) * CH)
                    ps = psp.tile([HID, CH], F32, name="ps", tag="ps")
                    for j in range(CH // MM):
                        ms = slice(j * MM, (j + 1) * MM)
                        nc.tensor.matmul(ps[:, ms], w0t[:],
                                         es[:, k * CH + j * MM:
                                             k * CH + (j + 1) * MM],
                                         start=True, stop=True)
                    nc.scalar.activation(os[:, cs], ps[:], AF.Relu, bias=b0t[:])
                nc.scalar.dma_start(out=out[:, s0:s0 + ssz], in_=os[:, :ssz])
    nc.finalize()
    return nc


def _build_step():
    """in: z [128, EC] f16 (rows 0:64 h^T, 64:128 agg1^T incl. W1b+b1)
       -> mt [64, EC] f16 = relu(relu(h@W1t + agg1) @ W2 + b2)^T
       wpk [128, 128] f16: cols 0:64 = [W1t; I], rows 0:64 cols 64:128 = W2"""
    nc = bacc.Bacc(trn_type="TRN2", num_devices=NCORES)
    z = nc.dram_tensor("z", [2 * HID, EC], F16, kind="ExternalInput")
    wpk = nc.dram_tensor("wpk", [2 * HID, 2 * HID], F16, kind="ExternalInput")
    b2c = nc.dram_tensor("b2c", [HID, 1], F32, kind="ExternalInput")
    out = nc.dram_tensor("out", [HID, EC], F16, kind="ExternalOutput")
    with tile.TileContext(nc) as tc:
        with (
            tc.tile_pool(name="const", bufs=1) as constp,
            tc.tile_pool(name="work", bufs=3) as workp,
            tc.tile_pool(name="ps1", bufs=2, space="PSUM") as ps1p,
            tc.tile_pool(name="ps2", bufs=2, space="PSUM") as ps2p,
        ):
            wt = constp.tile([2 * HID, 2 * HID], F16, name="wt")
            nc.sync.dma_start(out=wt[:], in_=wpk[:])
            b2t = constp.tile([HID, 1], F32, name="b2t")
            nc.scalar.dma_start(out=b2t[:], in_=b2c[:])
            for s0, ssz in _slabs(EC):
                zs = workp.tile([2 * HID, SLAB], F16, name="zs", tag="zs")
                nc.sync.dma_start(out=zs[:, :ssz], in_=z[:, s0:s0 + ssz])
                os = workp.tile([HID, SLAB], F16, name="os", tag="os")
                for k in range(ssz // CH):
                    cs = slice(k * CH, (k + 1) * CH)
                    ps1 = ps1p.tile([HID, CH], F32, name="ps1", tag="ps1")
                    for j in range(CH // MM):
                        nc.tensor.matmul(
                            ps1[:, j * MM:(j + 1) * MM], wt[:, 0:HID],
                            zs[:, k * CH + j * MM:k * CH + (j + 1) * MM],
                            start=True, stop=True)
                    m1 = workp.tile([HID, CH], F16, name="m1", tag="m1")
                    nc.scalar.activation(m1[:], ps1[:], AF.Relu)
                    ps2 = ps2p.tile([HID, CH], F32, name="ps2", tag="ps2")
                    for j in range(CH // MM):
                        nc.tensor.matmul(
                            ps2[:, j * MM:(j + 1) * MM],
                            wt[0:HID, HID:2 * HID],
                            m1[:, j * MM:(j + 1) * MM],
                            start=True, stop=True)
                    nc.vector.tensor_scalar(os[:, cs], ps2[:], b2t[:], 0.0,
                                            ALU.add, ALU.max)
                nc.scalar.dma_start(out=out[:, s0:s0 + ssz], in_=os[:, :ssz])
    nc.finalize()
    return nc


def _build_readout():
    """in: xt [128, NC_SH] f16, nst [64, NC_SH] f16
       -> ot [64, NC_SH] f32 = (relu([x,ns]@W3+b3) @ W4 + b4)^T
       wpk [128, 192] f16: cols 0:64 = W3x, rows 0:64 cols 64:128 = W3m,
       rows 0:64 cols 128:192 = W4.  bpk [64, 2] f32: col 0 = b3, col 1 = b4."""
    nc = bacc.Bacc(trn_type="TRN2", num_devices=NCORES)
    xt = nc.dram_tensor("xt", [128, NC_SH], F16, kind="ExternalInput")
    nst = nc.dram_tensor("nst", [HID, NC_SH], F16, kind="ExternalInput")
    wpk = nc.dram_tensor("wpk", [128, 192], F16, kind="ExternalInput")
    bpk = nc.dram_tensor("bpk", [HID, 2], F32, kind="ExternalInput")
    out = nc.dram_tensor("out", [HID, NC_SH], F32, kind="ExternalOutput")
    with tile.TileContext(nc) as tc:
        with (
            tc.tile_pool(name="const", bufs=1) as constp,
            tc.tile_pool(name="work", bufs=3) as workp,
            tc.tile_pool(name="ps1", bufs=2, space="PSUM") as ps1p,
            tc.tile_pool(name="ps2", bufs=2, space="PSUM") as ps2p,
        ):
            wt = constp.tile([128, 192], F16, name="wt")
            nc.sync.dma_start(out=wt[:], in_=wpk[:])
            bt = constp.tile([HID, 2], F32, name="bt")
            nc.scalar.dma_start(out=bt[:], in_=bpk[:])
            xs = constp.tile([128, NC_SH], F16, name="xs")
            nc.sync.dma_start(out=xs[:], in_=xt[:])
            nss = constp.tile([HID, NC_SH], F16, name="nss")
            nc.scalar.dma_start(out=nss[:], in_=nst[:])
            os = constp.tile([HID, NC_SH], F32, name="osr")
            for k in range(NC_SH // CH):
                cs = slice(k * CH, (k + 1) * CH)
                ps1 = ps1p.tile([HID, CH], F32, name="ps1", tag="ps1")
                for j in range(CH // MM):
                    ms = slice(k * CH + j * MM, k * CH + (j + 1) * MM)
                    pj = ps1[:, j * MM:(j + 1) * MM]
                    nc.tensor.matmul(pj, wt[:, 0:HID], xs[:, ms],
                                     start=True, stop=False)
                    nc.tensor.matmul(pj, wt[0:HID, HID:128], nss[:, ms],
                                     start=False, stop=True)
                r1 = workp.tile([HID, CH], F16, name="r1", tag="r1")
                nc.scalar.activation(r1[:], ps1[:], AF.Relu, bias=bt[:, 0:1])
                ps2 = ps2p.tile([HID, CH], F32, name="ps2", tag="ps2")
                for j in range(CH // MM):
                    nc.tensor.matmul(ps2[:, j * MM:(j + 1) * MM],
                                     wt[0:HID, 128:192],
                                     r1[:, j * MM:(j + 1) * MM],
                                     start=True, stop=True)
                nc.vector.tensor_scalar(os[:, cs], ps2[:], bt[:, 1:2], None,
                                        ALU.add)
            nc.scalar.dma_start(out=out[:], in_=os[:])
    nc.finalize()
    return nc


_CACHE = {}


def _get(name, builder):
    if name not in _CACHE:
        _CACHE[name] = builder()
    return _CACHE[name]


def _run_spmd(nc, per_core, shared, trace=False):
    """per_core: dict name -> full array sliced [.., c*W:(c+1)*W] along axis 1;
       shared: dict name -> replicated array."""
    in_maps = []
    for c in range(NCORES):
        m = {}
        for k, v in per_core.items():
            w = v.shape[1] // NCORES
            m[k] = np.ascontiguousarray(v[:, c * w:(c + 1) * w])
        m.update(shared)
        in_maps.append(m)
    res = run_bass_kernel_spmd(nc, in_maps, list(range(NCORES)), trace=trace)
    outs = np.concatenate([res.results[c]["out"] for c in range(NCORES)], axis=1)
    return outs, res


def kernel(**inputs):
    x = np.asarray(inputs["x"], np.float32)
    edge_index = np.asarray(inputs["edge_index"])
    ea = np.asarray(inputs["edge_attr"], np.float32)
    W0, b0, W1, b1, W2, b2, W3, b3, W4, b4 = (
        np.asarray(inputs[k], np.float32) for k in
        ["W0", "b0", "W1", "b1", "W2", "b2", "W3", "b3", "W4", "b4"])
    src = edge_index[0].astype(np.int64)
    tgt = edge_index[1].astype(np.int64)
    E = src.shape[0]
    W1t, W1b = W1[:HID], W1[HID:]

    # reverse-pair structure (exactly the reference's construction)
    key = src * N_NODES + tgt
    order = np.argsort(key, kind="stable")
    key_sorted = key[order]
    rev_key = tgt * N_NODES + src
    lo = np.searchsorted(key_sorted, rev_key, side="left")
    hi = np.searchsorted(key_sorted, rev_key, side="right")
    special = np.nonzero(hi > lo)[0]

    # tgt-sorted order for fast host segment sums
    torder = np.argsort(tgt, kind="stable")
    tsorted = tgt[torder]
    seg_starts = np.searchsorted(tsorted, np.arange(N_NODES), side="left")
    reduce_idx = np.minimum(seg_starts, E - 1)
    empty = seg_starts == np.append(seg_starts[1:], E)

    def segsum_T(hT):
        # hT [64, E] -> ns [N, 64]
        s = np.add.reduceat(hT[:, torder], reduce_idx, axis=1)
        s[:, empty] = 0.0
        return np.ascontiguousarray(s.T)

    EPAD = NCORES * EC

    # ---- h0 on device ----
    nc_first = _get("first", _build_first)
    ea_t = np.zeros((EDGE_F, EPAD), np.float16)
    ea_t[:, :E] = ea.T
    h0t, _ = _run_spmd(nc_first, {"ea_t": ea_t},
                       {"w0": W0.astype(np.float16),
                        "b0c": b0.reshape(HID, 1).astype(np.float32)})
    hT = h0t[:, :E].astype(np.float32)

    # ---- steps ----
    nc_step = _get("step", _build_step)
    wpk = np.zeros((2 * HID, 2 * HID), np.float16)
    wpk[:HID, :HID] = W1t.astype(np.float16)
    wpk[HID:, :HID] = np.eye(HID, dtype=np.float16)
    wpk[:HID, HID:] = W2.astype(np.float16)
    b2c = b2.reshape(HID, 1).astype(np.float32)
    for _ in range(STEPS):
        ns = segsum_T(hT)
        t = ns @ W1b + b1
        agg1 = t[src]
        for e in special:
            rev = hT[:, order[lo[e]:hi[e]]].sum(axis=1)
            agg1[e] = (ns[src[e]] - rev) @ W1b + b1
        z = np.zeros((2 * HID, EPAD), np.float16)
        z[:HID, :E] = hT
        z[HID:, :E] = agg1.T
        mt, _ = _run_spmd(nc_step, {"z": z}, {"wpk": wpk, "b2c": b2c})
        hT += mt[:, :E].astype(np.float32)

    # ---- readout on device ----
    nc_ro = _get("readout", _build_readout)
    ns = segsum_T(hT)
    NSH = N_NODES // NCORES
    NPAD = NCORES * NC_SH
    xt = np.zeros((128, NPAD), np.float16)
    nst = np.zeros((HID, NPAD), np.float16)
    for c in range(NCORES):
        xt[:, c * NC_SH:c * NC_SH + NSH] = x[c * NSH:(c + 1) * NSH].T
        nst[:, c * NC_SH:c * NC_SH + NSH] = ns[c * NSH:(c + 1) * NSH].T
    wpk_ro = np.zeros((128, 192), np.float16)
    wpk_ro[:, :HID] = W3[:128].astype(np.float16)
    wpk_ro[:HID, HID:128] = W3[128:].astype(np.float16)
    wpk_ro[:HID, 128:] = W4.astype(np.float16)
    bpk = np.stack([b3, b4], axis=1).astype(np.float32)
    ot, _ = _run_spmd(nc_ro, {"xt": xt, "nst": nst},
                      {"wpk": wpk_ro, "bpk": bpk})
    outs = [ot[:, c * NC_SH:c * NC_SH + NSH].T for c in range(NCORES)]
    return np.ascontiguousarray(np.concatenate(outs, axis=0), dtype=np.float32)


# revision 27
# speedup vs baseline: 1.9062x; 1.1677x over previous
# Trainium2 Bass kernel for nn_DirectedMessagePassing (chemprop-style DMPNN).
#
# Device executes all dense compute (per-edge MLPs and readout MLP) as
# Bass/Tile kernels SPMD across the 8 NeuronCores, edges/nodes sharded 1/8
# per core.  Index-only segment-sum / gather plumbing between the dense
# stages runs on the host.
#
# All device I/O is fp16 in transposed [feat, elem] layout, batched into
# 4096-elem DMA slabs (8KB/partition; HWDGE fixed overhead ~650ns per DMA
# instruction would otherwise dominate).  Matmuls run in fp16 at the
# 512-col moving-operand max, two matmuls per [64, 1024] PSUM tile so the
# activation / bias ops run at 1024 cols per instruction.  Biases are folded
# into activation/vector ops.
#
#   h0 = relu(ea @ W0 + b0)                          (device, edge-sharded)
#   per step: ns = segment_sum(h by tgt); t = ns @ W1b + b1   (host)
#             agg1[e] = t[src[e]] - rev corrections          (host gather)
#             m = relu(relu(h@W1t + agg1) @ W2 + b2)          (device)
#             h += m                                          (host add)
#   out = relu([x, segsum(h)] @ W3 + b3) @ W4 + b4    (device, node-sharded)
import numpy as np

import concourse.bacc as bacc
import concourse.mybir as mybir
import concourse.tile as tile
from concourse.bass_utils import run_bass_kernel_spmd

N_NODES = 40000
N_EDGES = 400000
EDGE_F = 16
HID = 64
STEPS = 3
NCORES = 8

F32 = mybir.dt.float32
F16 = mybir.dt.float16
AF = mybir.ActivationFunctionType
ALU = mybir.AluOpType

MM = 512            # free-dim cols per matmul (fp16 moving-operand max)
CH = 1024           # cols per PSUM half-tile
SLAB = 4096         # elems per DMA instruction (8KB/partition in fp16)
EC = 51200          # padded edges per core (12*4096 + 2048; even chunks/slab)
EC2 = EC // 2       # interleaved-output width
NC_SH = 5120        # padded nodes per core


def _slabs(total):
    s = 0
    while s < total:
        sz = min(SLAB, total - s)
        yield s, sz
        s += sz


def _build_first():
    """in: ea_t [16, EC] f16 -> out [128, EC2] f16: chunk-pair interleaved
    relu(ea@W0+b0)^T — even 1024-col chunk on partitions 0:64, odd on 64:128.
    b0d is b0 stacked twice: [128, 1] f32."""
    nc = bacc.Bacc(trn_type="TRN2", num_devices=NCORES)
    ea_t = nc.dram_tensor("ea_t", [EDGE_F, EC], F16, kind="ExternalInput")
    w0 = nc.dram_tensor("w0", [EDGE_F, HID], F16, kind="ExternalInput")
    b0d = nc.dram_tensor("b0d", [2 * HID, 1], F32, kind="ExternalInput")
    out = nc.dram_tensor("out", [2 * HID, EC2], F16, kind="ExternalOutput")
    with tile.TileContext(nc) as tc:
        with (
            tc.tile_pool(name="const", bufs=1) as constp,
            tc.tile_pool(name="work", bufs=3) as workp,
            tc.tile_pool(name="ps", bufs=2, space="PSUM") as psp,
        ):
            w0t = constp.tile([EDGE_F, HID], F16, name="w0t")
            nc.sync.dma_start(out=w0t[:], in_=w0[:])
            b0t = constp.tile([2 * HID, 1], F32, name="b0t")
            nc.scalar.dma_start(out=b0t[:], in_=b0d[:])
            for s0, ssz in _slabs(EC):
                es = workp.tile([EDGE_F, SLAB], F16, name="es", tag="es")
                nc.sync.dma_start(out=es[:, :ssz], in_=ea_t[:, s0:s0 + ssz])
                os = workp.tile([2 * HID, SLAB // 2], F16, name="os", tag="os")
                for p in range(ssz // (2 * CH)):    # chunk pair -> 128 parts
                    cs = slice(p * CH, (p + 1) * CH)
                    ps = psp.tile([2 * HID, CH], F32, name="ps", tag="ps")
                    for half in range(2):
                        pr = slice(half * HID, half * HID + HID)
                        c0 = (2 * p + half) * CH
                        for j in range(CH // MM):
                            nc.tensor.matmul(
                                ps[pr, j * MM:(j + 1) * MM], w0t[:],
                                es[:, c0 + j * MM:c0 + (j + 1) * MM],
                                start=True, stop=True)
                    nc.scalar.activation(os[:, cs], ps[:], AF.Relu, bias=b0t[:])
                nc.scalar.dma_start(out=out[:, s0 // 2:(s0 + ssz) // 2],
                                    in_=os[:, :ssz // 2])
    nc.finalize()
    return nc


def _build_step():
    """in: z [128, EC] f16 (rows 0:64 h^T, 64:128 agg1^T incl. W1b+b1)
       -> out [128, EC2] f16: chunk-pair interleaved
       relu(relu(h@W1t + agg1) @ W2 + b2)^T
       wpk [128, 128] f16: cols 0:64 = [W1t; I], cols 64:128 = W2 in both
       row halves.  b2d [128, 1] f32 = b2 stacked twice."""
    nc = bacc.Bacc(trn_type="TRN2", num_devices=NCORES)
    z = nc.dram_tensor("z", [2 * HID, EC], F16, kind="ExternalInput")
    wpk = nc.dram_tensor("wpk", [2 * HID, 2 * HID], F16, kind="ExternalInput")
    b2d = nc.dram_tensor("b2d", [2 * HID, 1], F32, kind="ExternalInput")
    out = nc.dram_tensor("out", [2 * HID, EC2], F16, kind="ExternalOutput")
    with tile.TileContext(nc) as tc:
        with (
            tc.tile_pool(name="const", bufs=1) as constp,
            tc.tile_pool(name="work", bufs=3) as workp,
            tc.tile_pool(name="ps1", bufs=2, space="PSUM") as ps1p,
            tc.tile_pool(name="ps2", bufs=2, space="PSUM") as ps2p,
        ):
            wt = constp.tile([2 * HID, 2 * HID], F16, name="wt")
            nc.sync.dma_start(out=wt[:], in_=wpk[:])
            b2t = constp.tile([2 * HID, 1], F32, name="b2t")
            nc.scalar.dma_start(out=b2t[:], in_=b2d[:])
            for s0, ssz in _slabs(EC):
                zs = workp.tile([2 * HID, SLAB], F16, name="zs", tag="zs")
                nc.sync.dma_start(out=zs[:, :ssz], in_=z[:, s0:s0 + ssz])
                os = workp.tile([2 * HID, SLAB // 2], F16, name="os", tag="os")
                for p in range(ssz // (2 * CH)):    # chunk pair -> 128 parts
                    cs = slice(p * CH, (p + 1) * CH)
                    ps1 = ps1p.tile([2 * HID, CH], F32, name="ps1", tag="ps1")
                    for half in range(2):
                        pr = slice(half * HID, half * HID + HID)
                        c0 = (2 * p + half) * CH
                        for j in range(CH // MM):
                            nc.tensor.matmul(
                                ps1[pr, j * MM:(j + 1) * MM], wt[:, 0:HID],
                                zs[:, c0 + j * MM:c0 + (j + 1) * MM],
                                start=True, stop=True)
                    m1 = workp.tile([2 * HID, CH], F16, name="m1", tag="m1")
                    nc.scalar.activation(m1[:], ps1[:], AF.Relu)
                    ps2 = ps2p.tile([2 * HID, CH], F32, name="ps2", tag="ps2")
                    for half in range(2):
                        pr = slice(half * HID, half * HID + HID)
                        for j in range(CH // MM):
                            nc.tensor.matmul(
                                ps2[pr, j * MM:(j + 1) * MM],
                                wt[pr, HID:2 * HID],
                                m1[pr, j * MM:(j + 1) * MM],
                                start=True, stop=True)
                    nc.vector.tensor_scalar(os[:, cs], ps2[:], b2t[:], 0.0,
                                            ALU.add, ALU.max)
                nc.scalar.dma_start(out=out[:, s0 // 2:(s0 + ssz) // 2],
                                    in_=os[:, :ssz // 2])
    nc.finalize()
    return nc


def _build_readout():
    """in: xt [128, NC_SH] f16, nst [64, NC_SH] f16
       -> ot [64, NC_SH] f32 = (relu([x,ns]@W3+b3) @ W4 + b4)^T
       wpk [128, 192] f16: cols 0:64 = W3x, rows 0:64 cols 64:128 = W3m,
       rows 0:64 cols 128:192 = W4.  bpk [64, 2] f32: col 0 = b3, col 1 = b4."""
    nc = bacc.Bacc(trn_type="TRN2", num_devices=NCORES)
    xt = nc.dram_tensor("xt", [128, NC_SH], F16, kind="ExternalInput")
    nst = nc.dram_tensor("nst", [HID, NC_SH], F16, kind="ExternalInput")
    wpk = nc.dram_tensor("wpk", [128, 192], F16, kind="ExternalInput")
    bpk = nc.dram_tensor("bpk", [HID, 2], F32, kind="ExternalInput")
    out = nc.dram_tensor("out", [HID, NC_SH], F32, kind="ExternalOutput")
    with tile.TileContext(nc) as tc:
        with (
            tc.tile_pool(name="const", bufs=1) as constp,
            tc.tile_pool(name="work", bufs=3) as workp,
            tc.tile_pool(name="ps1", bufs=2, space="PSUM") as ps1p,
            tc.tile_pool(name="ps2", bufs=2, space="PSUM") as ps2p,
        ):
            wt = constp.tile([128, 192], F16, name="wt")
            nc.sync.dma_start(out=wt[:], in_=wpk[:])
            bt = constp.tile([HID, 2], F32, name="bt")
            nc.scalar.dma_start(out=bt[:], in_=bpk[:])
            xs = constp.tile([128, NC_SH], F16, name="xs")
            nc.sync.dma_start(out=xs[:], in_=xt[:])
            nss = constp.tile([HID, NC_SH], F16, name="nss")
            nc.scalar.dma_start(out=nss[:], in_=nst[:])
            os = constp.tile([HID, NC_SH], F32, name="osr")
            for k in range(NC_SH // CH):
                cs = slice(k * CH, (k + 1) * CH)
                ps1 = ps1p.tile([HID, CH], F32, name="ps1", tag="ps1")
                for j in range(CH // MM):
                    ms = slice(k * CH + j * MM, k * CH + (j + 1) * MM)
                    pj = ps1[:, j * MM:(j + 1) * MM]
                    nc.tensor.matmul(pj, wt[:, 0:HID], xs[:, ms],
                                     start=True, stop=False)
                    nc.tensor.matmul(pj, wt[0:HID, HID:128], nss[:, ms],
                                     start=False, stop=True)
                r1 = workp.tile([HID, CH], F16, name="r1", tag="r1")
                nc.scalar.activation(r1[:], ps1[:], AF.Relu, bias=bt[:, 0:1])
                ps2 = ps2p.tile([HID, CH], F32, name="ps2", tag="ps2")
                for j in range(CH // MM):
                    nc.tensor.matmul(ps2[:, j * MM:(j + 1) * MM],
                                     wt[0:HID, 128:192],
                                     r1[:, j * MM:(j + 1) * MM],
                                     start=True, stop=True)
                nc.vector.tensor_scalar(os[:, cs], ps2[:], bt[:, 1:2], None,
                                        ALU.add)
                if k % 2 == 1 or k == NC_SH // CH - 1:
                    o0 = (k // 2) * 2 * CH
                    nc.scalar.dma_start(out=out[:, o0:(k + 1) * CH],
                                        in_=os[:, o0:(k + 1) * CH])
    nc.finalize()
    return nc


_CACHE = {}


def _get(name, builder):
    if name not in _CACHE:
        _CACHE[name] = builder()
    return _CACHE[name]


def _deinterleave(o):
    """[128, NCORES*EC2] chunk-pair interleaved -> [64, NCORES*EC] m^T."""
    mt = np.empty((HID, NCORES * EC), o.dtype)
    for c in range(NCORES):
        oc = o[:, c * EC2:(c + 1) * EC2]
        for s0, ssz in _slabs(EC):
            blk = oc[:, s0 // 2:(s0 + ssz) // 2]
            nq = ssz // (2 * CH)
            top = blk[:HID].reshape(HID, nq, CH)
            bot = blk[HID:].reshape(HID, nq, CH)
            mt[:, c * EC + s0:c * EC + s0 + ssz] = np.stack(
                [top, bot], axis=2).reshape(HID, ssz)
    return mt


def _interleave(mt):
    """[64, NCORES*EC] -> [128, NCORES*EC2] chunk-pair interleaved."""
    o = np.empty((2 * HID, NCORES * EC2), mt.dtype)
    for c in range(NCORES):
        for s0, ssz in _slabs(EC):
            nq = ssz // (2 * CH)
            blk = mt[:, c * EC + s0:c * EC + s0 + ssz].reshape(HID, nq, 2, CH)
            o[:HID, c * EC2 + s0 // 2:c * EC2 + (s0 + ssz) // 2] = \
                blk[:, :, 0].reshape(HID, nq * CH)
            o[HID:, c * EC2 + s0 // 2:c * EC2 + (s0 + ssz) // 2] = \
                blk[:, :, 1].reshape(HID, nq * CH)
    return o


def _run_spmd(nc, per_core, shared, trace=False):
    """per_core: dict name -> full array sliced [.., c*W:(c+1)*W] along axis 1;
       shared: dict name -> replicated array."""
    in_maps = []
    for c in range(NCORES):
        m = {}
        for k, v in per_core.items():
            w = v.shape[1] // NCORES
            m[k] = np.ascontiguousarray(v[:, c * w:(c + 1) * w])
        m.update(shared)
        in_maps.append(m)
    res = run_bass_kernel_spmd(nc, in_maps, list(range(NCORES)), trace=trace)
    outs = np.concatenate([res.results[c]["out"] for c in range(NCORES)], axis=1)
    return outs, res


def kernel(**inputs):
    x = np.asarray(inputs["x"], np.float32)
    edge_index = np.asarray(inputs["edge_index"])
    ea = np.asarray(inputs["edge_attr"], np.float32)
    W0, b0, W1, b1, W2, b2, W3, b3, W4, b4 = (
        np.asarray(inputs[k], np.float32) for k in
        ["W0", "b0", "W1", "b1", "W2", "b2", "W3", "b3", "W4", "b4"])
    src = edge_index[0].astype(np.int64)
    tgt = edge_index[1].astype(np.int64)
    E = src.shape[0]
    W1t, W1b = W1[:HID], W1[HID:]

    # reverse-pair structure (exactly the reference's construction)
    key = src * N_NODES + tgt
    order = np.argsort(key, kind="stable")
    key_sorted = key[order]
    rev_key = tgt * N_NODES + src
    lo = np.searchsorted(key_sorted, rev_key, side="left")
    hi = np.searchsorted(key_sorted, rev_key, side="right")
    special = np.nonzero(hi > lo)[0]

    # tgt-sorted order for fast host segment sums
    torder = np.argsort(tgt, kind="stable")
    tsorted = tgt[torder]
    seg_starts = np.searchsorted(tsorted, np.arange(N_NODES), side="left")
    reduce_idx = np.minimum(seg_starts, E - 1)
    empty = seg_starts == np.append(seg_starts[1:], E)

    def segsum_T(hT):
        # hT [64, E] -> ns [N, 64]
        s = np.add.reduceat(hT[:, torder], reduce_idx, axis=1)
        s[:, empty] = 0.0
        return np.ascontiguousarray(s.T)

    EPAD = NCORES * EC

    # ---- h0 on device ----
    nc_first = _get("first", _build_first)
    ea_t = np.zeros((EDGE_F, EPAD), np.float16)
    ea_t[:, :E] = ea.T
    h0t, _ = _run_spmd(nc_first, {"ea_t": ea_t},
                       {"w0": W0.astype(np.float16),
                        "b0d": np.concatenate([b0, b0]).reshape(
                            2 * HID, 1).astype(np.float32)})
    hT = _deinterleave(h0t)[:, :E].astype(np.float32)

    # ---- steps ----
    nc_step = _get("step", _build_step)
    wpk = np.zeros((2 * HID, 2 * HID), np.float16)
    wpk[:HID, :HID] = W1t.astype(np.float16)
    wpk[HID:, :HID] = np.eye(HID, dtype=np.float16)
    wpk[:HID, HID:] = W2.astype(np.float16)
    wpk[HID:, HID:] = W2.astype(np.float16)
    b2d = np.concatenate([b2, b2]).reshape(2 * HID, 1).astype(np.float32)
    for _ in range(STEPS):
        ns = segsum_T(hT)
        t = ns @ W1b + b1
        agg1 = t[src]
        for e in special:
            rev = hT[:, order[lo[e]:hi[e]]].sum(axis=1)
            agg1[e] = (ns[src[e]] - rev) @ W1b + b1
        z = np.zeros((2 * HID, EPAD), np.float16)
        z[:HID, :E] = hT
        z[HID:, :E] = agg1.T
        mt, _ = _run_spmd(nc_step, {"z": z}, {"wpk": wpk, "b2d": b2d})
        hT += _deinterleave(mt)[:, :E].astype(np.float32)

    # ---- readout on device ----
    nc_ro = _get("readout", _build_readout)
    ns = segsum_T(hT)
    NSH = N_NODES // NCORES
    NPAD = NCORES * NC_SH
    xt = np.zeros((128, NPAD), np.float16)
    nst = np.zeros((HID, NPAD), np.float16)
    for c in range(NCORES):
        xt[:, c * NC_SH:c * NC_SH + NSH] = x[c * NSH:(c + 1) * NSH].T
        nst[:, c * NC_SH:c * NC_SH + NSH] = ns[c * NSH:(c + 1) * NSH].T
    wpk_ro = np.zeros((128, 192), np.float16)
    wpk_ro[:, :HID] = W3[:128].astype(np.float16)
    wpk_ro[:HID, HID:128] = W3[128:].astype(np.float16)
    wpk_ro[:HID, 128:] = W4.astype(np.float16)
    bpk = np.stack([b3, b4], axis=1).astype(np.float32)
    ot, _ = _run_spmd(nc_ro, {"xt": xt, "nst": nst},
                      {"wpk": wpk_ro, "bpk": bpk})
    outs = [ot[:, c * NC_SH:c * NC_SH + NSH].T for c in range(NCORES)]
    return np.ascontiguousarray(np.concatenate(outs, axis=0), dtype=np.float32)


# revision 28
# speedup vs baseline: 2.1204x; 1.1124x over previous
# Trainium2 Bass kernel for nn_DirectedMessagePassing (chemprop-style DMPNN).
#
# Device executes all dense compute (per-edge MLPs and readout MLP) as
# Bass/Tile kernels SPMD across the 8 NeuronCores, edges/nodes sharded 1/8
# per core.  Index-only segment-sum / gather plumbing between the dense
# stages runs on the host.
#
# All device I/O is fp16 in transposed [feat, elem] layout, batched into
# 4096-elem DMA slabs (8KB/partition; HWDGE fixed overhead ~650ns per DMA
# instruction would otherwise dominate).  Matmuls run in fp16 at the
# 512-col moving-operand max, two matmuls per [64, 1024] PSUM tile so the
# activation / bias ops run at 1024 cols per instruction.  Biases are folded
# into activation/vector ops.
#
#   h0 = relu(ea @ W0 + b0)                          (device, edge-sharded)
#   per step: ns = segment_sum(h by tgt); t = ns @ W1b + b1   (host)
#             agg1[e] = t[src[e]] - rev corrections          (host gather)
#             m = relu(relu(h@W1t + agg1) @ W2 + b2)          (device)
#             h += m                                          (host add)
#   out = relu([x, segsum(h)] @ W3 + b3) @ W4 + b4    (device, node-sharded)
import numpy as np

import concourse.bacc as bacc
import concourse.mybir as mybir
import concourse.tile as tile
from concourse.bass_utils import run_bass_kernel_spmd

N_NODES = 40000
N_EDGES = 400000
EDGE_F = 16
HID = 64
STEPS = 3
NCORES = 8

F32 = mybir.dt.float32
F16 = mybir.dt.float16
AF = mybir.ActivationFunctionType
ALU = mybir.AluOpType

MM = 512            # free-dim cols per matmul (fp16 moving-operand max)
CH = 1024           # cols per PSUM half-tile
SLAB = 4096         # elems per DMA instruction (8KB/partition in fp16)
EC = 51200          # padded edges per core (12*4096 + 2048; even chunks/slab)
EC2 = EC // 2       # interleaved-output width
NC_SH = 5120        # padded nodes per core


def _slabs(total):
    s = 0
    while s < total:
        sz = min(SLAB, total - s)
        yield s, sz
        s += sz


def _build_first():
    """in: ea_t [16, EC] f16 -> out [128, EC2] f16: chunk-pair interleaved
    relu(ea@W0+b0)^T — even 1024-col chunk on partitions 0:64, odd on 64:128.
    b0d is b0 stacked twice: [128, 1] f32."""
    nc = bacc.Bacc(trn_type="TRN2", num_devices=NCORES)
    ea_t = nc.dram_tensor("ea_t", [EDGE_F, EC], F16, kind="ExternalInput")
    w0 = nc.dram_tensor("w0", [EDGE_F, HID], F16, kind="ExternalInput")
    b0d = nc.dram_tensor("b0d", [2 * HID, 1], F32, kind="ExternalInput")
    out = nc.dram_tensor("out", [2 * HID, EC2], F16, kind="ExternalOutput")
    with tile.TileContext(nc) as tc:
        with (
            tc.tile_pool(name="const", bufs=1) as constp,
            tc.tile_pool(name="work", bufs=3) as workp,
            tc.tile_pool(name="ps", bufs=2, space="PSUM") as psp,
        ):
            w0t = constp.tile([EDGE_F, HID], F16, name="w0t")
            nc.sync.dma_start(out=w0t[:], in_=w0[:])
            b0t = constp.tile([2 * HID, 1], F32, name="b0t")
            nc.scalar.dma_start(out=b0t[:], in_=b0d[:])
            for s0, ssz in _slabs(EC):
                es = workp.tile([EDGE_F, SLAB], F16, name="es", tag="es")
                nc.sync.dma_start(out=es[:, :ssz], in_=ea_t[:, s0:s0 + ssz])
                os = workp.tile([2 * HID, SLAB // 2], F16, name="os", tag="os")
                for p in range(ssz // (2 * CH)):    # chunk pair -> 128 parts
                    cs = slice(p * CH, (p + 1) * CH)
                    ps = psp.tile([2 * HID, CH], F32, name="ps", tag="ps")
                    for half in range(2):
                        pr = slice(half * HID, half * HID + HID)
                        c0 = (2 * p + half) * CH
                        for j in range(CH // MM):
                            nc.tensor.matmul(
                                ps[pr, j * MM:(j + 1) * MM], w0t[:],
                                es[:, c0 + j * MM:c0 + (j + 1) * MM],
                                start=True, stop=True)
                    nc.scalar.activation(os[:, cs], ps[:], AF.Relu, bias=b0t[:])
                nc.scalar.dma_start(out=out[:, s0 // 2:(s0 + ssz) // 2],
                                    in_=os[:, :ssz // 2])
    nc.finalize()
    return nc


def _build_step():
    """in: z [128, EC] f16 (rows 0:64 h^T, 64:128 agg1^T incl. W1b+b1)
       -> out [128, EC2] f16: chunk-pair interleaved
       relu(relu(h@W1t + agg1) @ W2 + b2)^T
       wpk [128, 256] f16: cols 0:64 = [W1t; I], cols 128:256 =
       block-diag(W2, W2) so layer2 runs as one K=128/M=128 matmul per
       512-col block.  b2d [128, 1] f32 = b2 stacked twice."""
    nc = bacc.Bacc(trn_type="TRN2", num_devices=NCORES)
    z = nc.dram_tensor("z", [2 * HID, EC], F16, kind="ExternalInput")
    wpk = nc.dram_tensor("wpk", [2 * HID, 256], F16, kind="ExternalInput")
    b2d = nc.dram_tensor("b2d", [2 * HID, 1], F32, kind="ExternalInput")
    out = nc.dram_tensor("out", [2 * HID, EC2], F16, kind="ExternalOutput")
    with tile.TileContext(nc) as tc:
        with (
            tc.tile_pool(name="const", bufs=1) as constp,
            tc.tile_pool(name="work", bufs=4) as workp,
            tc.tile_pool(name="ps1", bufs=2, space="PSUM") as ps1p,
            tc.tile_pool(name="ps2", bufs=2, space="PSUM") as ps2p,
        ):
            wt = constp.tile([2 * HID, 256], F16, name="wt")
            nc.sync.dma_start(out=wt[:], in_=wpk[:])
            b2t = constp.tile([2 * HID, 1], F32, name="b2t")
            nc.scalar.dma_start(out=b2t[:], in_=b2d[:])
            for s0, ssz in _slabs(EC):
                zs = workp.tile([2 * HID, SLAB], F16, name="zs", tag="zs")
                nc.sync.dma_start(out=zs[:, :ssz], in_=z[:, s0:s0 + ssz])
                os = workp.tile([2 * HID, SLAB // 2], F16, name="os", tag="os")
                for p in range(ssz // (2 * CH)):    # chunk pair -> 128 parts
                    cs = slice(p * CH, (p + 1) * CH)
                    ps1 = ps1p.tile([2 * HID, CH], F32, name="ps1", tag="ps1")
                    for half in range(2):
                        pr = slice(half * HID, half * HID + HID)
                        c0 = (2 * p + half) * CH
                        for j in range(CH // MM):
                            nc.tensor.matmul(
                                ps1[pr, j * MM:(j + 1) * MM], wt[:, 0:HID],
                                zs[:, c0 + j * MM:c0 + (j + 1) * MM],
                                start=True, stop=True)
                    m1 = workp.tile([2 * HID, CH], F16, name="m1", tag="m1")
                    nc.scalar.activation(m1[:], ps1[:], AF.Relu)
                    ps2 = ps2p.tile([2 * HID, CH], F32, name="ps2", tag="ps2")
                    for j in range(CH // MM):
                        nc.tensor.matmul(
                            ps2[:, j * MM:(j + 1) * MM], wt[:, 128:256],
                            m1[:, j * MM:(j + 1) * MM],
                            start=True, stop=True)
                    nc.vector.tensor_scalar(os[:, cs], ps2[:], b2t[:], 0.0,
                                            ALU.add, ALU.max)
                nc.scalar.dma_start(out=out[:, s0 // 2:(s0 + ssz) // 2],
                                    in_=os[:, :ssz // 2])
    nc.finalize()
    return nc


def _build_readout():
    """in: xt [128, NC_SH] f16, nst [64, NC_SH] f16
       -> ot [64, NC_SH] f32 = (relu([x,ns]@W3+b3) @ W4 + b4)^T
       wpk [128, 192] f16: cols 0:64 = W3x, rows 0:64 cols 64:128 = W3m,
       rows 0:64 cols 128:192 = W4.  bpk [64, 2] f32: col 0 = b3, col 1 = b4."""
    nc = bacc.Bacc(trn_type="TRN2", num_devices=NCORES)
    xt = nc.dram_tensor("xt", [128, NC_SH], F16, kind="ExternalInput")
    nst = nc.dram_tensor("nst", [HID, NC_SH], F16, kind="ExternalInput")
    wpk = nc.dram_tensor("wpk", [128, 192], F16, kind="ExternalInput")
    bpk = nc.dram_tensor("bpk", [HID, 2], F32, kind="ExternalInput")
    out = nc.dram_tensor("out", [HID, NC_SH], F32, kind="ExternalOutput")
    with tile.TileContext(nc) as tc:
        with (
            tc.tile_pool(name="const", bufs=1) as constp,
            tc.tile_pool(name="work", bufs=3) as workp,
            tc.tile_pool(name="ps1", bufs=2, space="PSUM") as ps1p,
            tc.tile_pool(name="ps2", bufs=2, space="PSUM") as ps2p,
        ):
            wt = constp.tile([128, 192], F16, name="wt")
            nc.sync.dma_start(out=wt[:], in_=wpk[:])
            bt = constp.tile([HID, 2], F32, name="bt")
            nc.scalar.dma_start(out=bt[:], in_=bpk[:])
            xs = constp.tile([128, NC_SH], F16, name="xs")
            nc.sync.dma_start(out=xs[:], in_=xt[:])
            nss = constp.tile([HID, NC_SH], F16, name="nss")
            nc.scalar.dma_start(out=nss[:], in_=nst[:])
            os = constp.tile([HID, NC_SH], F32, name="osr")
            for k in range(NC_SH // CH):
                cs = slice(k * CH, (k + 1) * CH)
                ps1 = ps1p.tile([HID, CH], F32, name="ps1", tag="ps1")
                for j in range(CH // MM):
                    ms = slice(k * CH + j * MM, k * CH + (j + 1) * MM)
                    pj = ps1[:, j * MM:(j + 1) * MM]
                    nc.tensor.matmul(pj, wt[:, 0:HID], xs[:, ms],
                                     start=True, stop=False)
                    nc.tensor.matmul(pj, wt[0:HID, HID:128], nss[:, ms],
                                     start=False, stop=True)
                r1 = workp.tile([HID, CH], F16, name="r1", tag="r1")
                nc.scalar.activation(r1[:], ps1[:], AF.Relu, bias=bt[:, 0:1])
                ps2 = ps2p.tile([HID, CH], F32, name="ps2", tag="ps2")
                for j in range(CH // MM):
                    nc.tensor.matmul(ps2[:, j * MM:(j + 1) * MM],
                                     wt[0:HID, 128:192],
                                     r1[:, j * MM:(j + 1) * MM],
                                     start=True, stop=True)
                nc.vector.tensor_scalar(os[:, cs], ps2[:], bt[:, 1:2], None,
                                        ALU.add)
                if k % 2 == 1 or k == NC_SH // CH - 1:
                    o0 = (k // 2) * 2 * CH
                    nc.scalar.dma_start(out=out[:, o0:(k + 1) * CH],
                                        in_=os[:, o0:(k + 1) * CH])
    nc.finalize()
    return nc


_CACHE = {}


def _get(name, builder):
    if name not in _CACHE:
        _CACHE[name] = builder()
    return _CACHE[name]


def _deinterleave(o):
    """[128, NCORES*EC2] chunk-pair interleaved -> [64, NCORES*EC] m^T."""
    mt = np.empty((HID, NCORES * EC), o.dtype)
    for c in range(NCORES):
        oc = o[:, c * EC2:(c + 1) * EC2]
        for s0, ssz in _slabs(EC):
            blk = oc[:, s0 // 2:(s0 + ssz) // 2]
            nq = ssz // (2 * CH)
            top = blk[:HID].reshape(HID, nq, CH)
            bot = blk[HID:].reshape(HID, nq, CH)
            mt[:, c * EC + s0:c * EC + s0 + ssz] = np.stack(
                [top, bot], axis=2).reshape(HID, ssz)
    return mt


def _interleave(mt):
    """[64, NCORES*EC] -> [128, NCORES*EC2] chunk-pair interleaved."""
    o = np.empty((2 * HID, NCORES * EC2), mt.dtype)
    for c in range(NCORES):
        for s0, ssz in _slabs(EC):
            nq = ssz // (2 * CH)
            blk = mt[:, c * EC + s0:c * EC + s0 + ssz].reshape(HID, nq, 2, CH)
            o[:HID, c * EC2 + s0 // 2:c * EC2 + (s0 + ssz) // 2] = \
                blk[:, :, 0].reshape(HID, nq * CH)
            o[HID:, c * EC2 + s0 // 2:c * EC2 + (s0 + ssz) // 2] = \
                blk[:, :, 1].reshape(HID, nq * CH)
    return o


def _run_spmd(nc, per_core, shared, trace=False):
    """per_core: dict name -> full array sliced [.., c*W:(c+1)*W] along axis 1;
       shared: dict name -> replicated array."""
    in_maps = []
    for c in range(NCORES):
        m = {}
        for k, v in per_core.items():
            w = v.shape[1] // NCORES
            m[k] = np.ascontiguousarray(v[:, c * w:(c + 1) * w])
        m.update(shared)
        in_maps.append(m)
    res = run_bass_kernel_spmd(nc, in_maps, list(range(NCORES)), trace=trace)
    outs = np.concatenate([res.results[c]["out"] for c in range(NCORES)], axis=1)
    return outs, res


def kernel(**inputs):
    x = np.asarray(inputs["x"], np.float32)
    edge_index = np.asarray(inputs["edge_index"])
    ea = np.asarray(inputs["edge_attr"], np.float32)
    W0, b0, W1, b1, W2, b2, W3, b3, W4, b4 = (
        np.asarray(inputs[k], np.float32) for k in
        ["W0", "b0", "W1", "b1", "W2", "b2", "W3", "b3", "W4", "b4"])
    src = edge_index[0].astype(np.int64)
    tgt = edge_index[1].astype(np.int64)
    E = src.shape[0]
    W1t, W1b = W1[:HID], W1[HID:]

    # reverse-pair structure (exactly the reference's construction)
    key = src * N_NODES + tgt
    order = np.argsort(key, kind="stable")
    key_sorted = key[order]
    rev_key = tgt * N_NODES + src
    lo = np.searchsorted(key_sorted, rev_key, side="left")
    hi = np.searchsorted(key_sorted, rev_key, side="right")
    special = np.nonzero(hi > lo)[0]

    # tgt-sorted order for fast host segment sums
    torder = np.argsort(tgt, kind="stable")
    tsorted = tgt[torder]
    seg_starts = np.searchsorted(tsorted, np.arange(N_NODES), side="left")
    reduce_idx = np.minimum(seg_starts, E - 1)
    empty = seg_starts == np.append(seg_starts[1:], E)

    def segsum_T(hT):
        # hT [64, E] -> ns [N, 64]
        s = np.add.reduceat(hT[:, torder], reduce_idx, axis=1)
        s[:, empty] = 0.0
        return np.ascontiguousarray(s.T)

    EPAD = NCORES * EC

    # ---- h0 on device ----
    nc_first = _get("first", _build_first)
    ea_t = np.zeros((EDGE_F, EPAD), np.float16)
    ea_t[:, :E] = ea.T
    h0t, _ = _run_spmd(nc_first, {"ea_t": ea_t},
                       {"w0": W0.astype(np.float16),
                        "b0d": np.concatenate([b0, b0]).reshape(
                            2 * HID, 1).astype(np.float32)})
    hT = _deinterleave(h0t)[:, :E].astype(np.float32)

    # ---- steps ----
    nc_step = _get("step", _build_step)
    wpk = np.zeros((2 * HID, 256), np.float16)
    wpk[:HID, :HID] = W1t.astype(np.float16)
    wpk[HID:, :HID] = np.eye(HID, dtype=np.float16)
    wpk[:HID, 2 * HID:3 * HID] = W2.astype(np.float16)
    wpk[HID:, 3 * HID:] = W2.astype(np.float16)
    b2d = np.concatenate([b2, b2]).reshape(2 * HID, 1).astype(np.float32)
    for _ in range(STEPS):
        ns = segsum_T(hT)
        t = ns @ W1b + b1
        agg1 = t[src]
        for e in special:
            rev = hT[:, order[lo[e]:hi[e]]].sum(axis=1)
            agg1[e] = (ns[src[e]] - rev) @ W1b + b1
        z = np.zeros((2 * HID, EPAD), np.float16)
        z[:HID, :E] = hT
        z[HID:, :E] = agg1.T
        mt, _ = _run_spmd(nc_step, {"z": z}, {"wpk": wpk, "b2d": b2d})
        hT += _deinterleave(mt)[:, :E].astype(np.float32)

    # ---- readout on device ----
    nc_ro = _get("readout", _build_readout)
    ns = segsum_T(hT)
    NSH = N_NODES // NCORES
    NPAD = NCORES * NC_SH
    xt = np.zeros((128, NPAD), np.float16)
    nst = np.zeros((HID, NPAD), np.float16)
    for c in range(NCORES):
        xt[:, c * NC_SH:c * NC_SH + NSH] = x[c * NSH:(c + 1) * NSH].T
        nst[:, c * NC_SH:c * NC_SH + NSH] = ns[c * NSH:(c + 1) * NSH].T
    wpk_ro = np.zeros((128, 192), np.float16)
    wpk_ro[:, :HID] = W3[:128].astype(np.float16)
    wpk_ro[:HID, HID:128] = W3[128:].astype(np.float16)
    wpk_ro[:HID, 128:] = W4.astype(np.float16)
    bpk = np.stack([b3, b4], axis=1).astype(np.float32)
    ot, _ = _run_spmd(nc_ro, {"xt": xt, "nst": nst},
                      {"wpk": wpk_ro, "bpk": bpk})
    outs = [ot[:, c * NC_SH:c * NC_SH + NSH].T for c in range(NCORES)]
    return np.ascontiguousarray(np.concatenate(outs, axis=0), dtype=np.float32)


# revision 29
# speedup vs baseline: 2.1535x; 1.0156x over previous
# Trainium2 Bass kernel for nn_DirectedMessagePassing (chemprop-style DMPNN).
#
# Device executes all dense compute (per-edge MLPs and readout MLP) as
# Bass/Tile kernels SPMD across the 8 NeuronCores, edges/nodes sharded 1/8
# per core.  Index-only segment-sum / gather plumbing between the dense
# stages runs on the host.
#
# All device I/O is fp16 in transposed [feat, elem] layout, batched into
# 4096-elem DMA slabs (8KB/partition; HWDGE fixed overhead ~650ns per DMA
# instruction would otherwise dominate).  Matmuls run in fp16 at the
# 512-col moving-operand max, two matmuls per [64, 1024] PSUM tile so the
# activation / bias ops run at 1024 cols per instruction.  Biases are folded
# into activation/vector ops.
#
#   h0 = relu(ea @ W0 + b0)                          (device, edge-sharded)
#   per step: ns = segment_sum(h by tgt); t = ns @ W1b + b1   (host)
#             agg1[e] = t[src[e]] - rev corrections          (host gather)
#             m = relu(relu(h@W1t + agg1) @ W2 + b2)          (device)
#             h += m                                          (host add)
#   out = relu([x, segsum(h)] @ W3 + b3) @ W4 + b4    (device, node-sharded)
import numpy as np

import concourse.bacc as bacc
import concourse.mybir as mybir
import concourse.tile as tile
from concourse.bass_utils import run_bass_kernel_spmd

N_NODES = 40000
N_EDGES = 400000
EDGE_F = 16
HID = 64
STEPS = 3
NCORES = 8

F32 = mybir.dt.float32
F16 = mybir.dt.float16
AF = mybir.ActivationFunctionType
ALU = mybir.AluOpType

MM = 512            # free-dim cols per matmul (fp16 moving-operand max)
CH = 1024           # cols per PSUM half-tile
SLAB = 4096         # elems per DMA instruction (8KB/partition in fp16)
EC = 51200          # padded edges per core (12*4096 + 2048; even chunks/slab)
EC2 = EC // 2       # interleaved-output width
NC_SH = 5120        # padded nodes per core


def _slabs(total):
    s = 0
    while s < total:
        sz = min(SLAB, total - s)
        yield s, sz
        s += sz


def _build_first():
    """in: ea_t [16, EC] f16 -> out [128, EC2] f16: chunk-pair interleaved
    relu(ea@W0+b0)^T — even 1024-col chunk on partitions 0:64, odd on 64:128.
    b0d is b0 stacked twice: [128, 1] f32."""
    nc = bacc.Bacc(trn_type="TRN2", num_devices=NCORES)
    ea_t = nc.dram_tensor("ea_t", [EDGE_F, EC], F16, kind="ExternalInput")
    w0 = nc.dram_tensor("w0", [EDGE_F, HID], F16, kind="ExternalInput")
    b0d = nc.dram_tensor("b0d", [2 * HID, 1], F32, kind="ExternalInput")
    out = nc.dram_tensor("out", [2 * HID, EC2], F16, kind="ExternalOutput")
    with tile.TileContext(nc) as tc:
        with (
            tc.tile_pool(name="const", bufs=1) as constp,
            tc.tile_pool(name="work", bufs=4) as workp,
            tc.tile_pool(name="ps", bufs=2, space="PSUM") as psp,
        ):
            w0t = constp.tile([EDGE_F, HID], F16, name="w0t")
            nc.sync.dma_start(out=w0t[:], in_=w0[:])
            b0t = constp.tile([2 * HID, 1], F32, name="b0t")
            nc.scalar.dma_start(out=b0t[:], in_=b0d[:])
            for s0, ssz in _slabs(EC):
                es = workp.tile([EDGE_F, SLAB], F16, name="es", tag="es")
                nc.sync.dma_start(out=es[:, :ssz], in_=ea_t[:, s0:s0 + ssz])
                os = workp.tile([2 * HID, SLAB // 2], F16, name="os", tag="os")
                for p in range(ssz // (2 * CH)):    # chunk pair -> 128 parts
                    cs = slice(p * CH, (p + 1) * CH)
                    ps = psp.tile([2 * HID, CH], F32, name="ps", tag="ps")
                    for half in range(2):
                        pr = slice(half * HID, half * HID + HID)
                        c0 = (2 * p + half) * CH
                        for j in range(CH // MM):
                            nc.tensor.matmul(
                                ps[pr, j * MM:(j + 1) * MM], w0t[:],
                                es[:, c0 + j * MM:c0 + (j + 1) * MM],
                                start=True, stop=True)
                    nc.scalar.activation(os[:, cs], ps[:], AF.Relu, bias=b0t[:])
                nc.scalar.dma_start(out=out[:, s0 // 2:(s0 + ssz) // 2],
                                    in_=os[:, :ssz // 2])
    nc.finalize()
    return nc


def _build_step():
    """in: z [128, EC] f16 (rows 0:64 h^T, 64:128 agg1^T incl. W1b+b1)
       -> out [128, EC2] f16: chunk-pair interleaved
       relu(relu(h@W1t + agg1) @ W2 + b2)^T
       wpk [128, 256] f16: cols 0:64 = [W1t; I], cols 128:256 =
       block-diag(W2, W2) so layer2 runs as one K=128/M=128 matmul per
       512-col block.  b2d [128, 1] f32 = b2 stacked twice."""
    nc = bacc.Bacc(trn_type="TRN2", num_devices=NCORES)
    z = nc.dram_tensor("z", [2 * HID, EC], F16, kind="ExternalInput")
    wpk = nc.dram_tensor("wpk", [2 * HID, 256], F16, kind="ExternalInput")
    b2d = nc.dram_tensor("b2d", [2 * HID, 1], F32, kind="ExternalInput")
    out = nc.dram_tensor("out", [2 * HID, EC2], F16, kind="ExternalOutput")
    with tile.TileContext(nc) as tc:
        with (
            tc.tile_pool(name="const", bufs=1) as constp,
            tc.tile_pool(name="work", bufs=4) as workp,
            tc.tile_pool(name="ps1", bufs=2, space="PSUM") as ps1p,
            tc.tile_pool(name="ps2", bufs=2, space="PSUM") as ps2p,
        ):
            wt = constp.tile([2 * HID, 256], F16, name="wt")
            nc.sync.dma_start(out=wt[:], in_=wpk[:])
            b2t = constp.tile([2 * HID, 1], F32, name="b2t")
            nc.scalar.dma_start(out=b2t[:], in_=b2d[:])
            for s0, ssz in _slabs(EC):
                zs = workp.tile([2 * HID, SLAB], F16, name="zs", tag="zs")
                nc.sync.dma_start(out=zs[:, :ssz], in_=z[:, s0:s0 + ssz])
                os = workp.tile([2 * HID, SLAB // 2], F16, name="os", tag="os")
                for p in range(ssz // (2 * CH)):    # chunk pair -> 128 parts
                    cs = slice(p * CH, (p + 1) * CH)
                    ps1 = ps1p.tile([2 * HID, CH], F32, name="ps1", tag="ps1")
                    for half in range(2):
                        pr = slice(half * HID, half * HID + HID)
                        c0 = (2 * p + half) * CH
                        for j in range(CH // MM):
                            nc.tensor.matmul(
                                ps1[pr, j * MM:(j + 1) * MM], wt[:, 0:HID],
                                zs[:, c0 + j * MM:c0 + (j + 1) * MM],
                                start=True, stop=True)
                    m1 = workp.tile([2 * HID, CH], F16, name="m1", tag="m1")
                    nc.scalar.activation(m1[:], ps1[:], AF.Relu)
                    ps2 = ps2p.tile([2 * HID, CH], F32, name="ps2", tag="ps2")
                    for j in range(CH // MM):
                        nc.tensor.matmul(
                            ps2[:, j * MM:(j + 1) * MM], wt[:, 128:256],
                            m1[:, j * MM:(j + 1) * MM],
                            start=True, stop=True)
                    nc.vector.tensor_scalar(os[:, cs], ps2[:], b2t[:], 0.0,
                                            ALU.add, ALU.max)
                nc.scalar.dma_start(out=out[:, s0 // 2:(s0 + ssz) // 2],
                                    in_=os[:, :ssz // 2])
    nc.finalize()
    return nc


def _build_readout():
    """in: xt [128, NC_SH] f16, nst [64, NC_SH] f16
       -> ot [64, NC_SH] f32 = (relu([x,ns]@W3+b3) @ W4 + b4)^T
       wpk [128, 192] f16: cols 0:64 = W3x, rows 0:64 cols 64:128 = W3m,
       rows 0:64 cols 128:192 = W4.  bpk [64, 2] f32: col 0 = b3, col 1 = b4."""
    nc = bacc.Bacc(trn_type="TRN2", num_devices=NCORES)
    xt = nc.dram_tensor("xt", [128, NC_SH], F16, kind="ExternalInput")
    nst = nc.dram_tensor("nst", [HID, NC_SH], F16, kind="ExternalInput")
    wpk = nc.dram_tensor("wpk", [128, 192], F16, kind="ExternalInput")
    bpk = nc.dram_tensor("bpk", [HID, 2], F32, kind="ExternalInput")
    out = nc.dram_tensor("out", [HID, NC_SH], F32, kind="ExternalOutput")
    with tile.TileContext(nc) as tc:
        with (
            tc.tile_pool(name="const", bufs=1) as constp,
            tc.tile_pool(name="work", bufs=3) as workp,
            tc.tile_pool(name="ps1", bufs=2, space="PSUM") as ps1p,
            tc.tile_pool(name="ps2", bufs=2, space="PSUM") as ps2p,
        ):
            wt = constp.tile([128, 192], F16, name="wt")
            nc.sync.dma_start(out=wt[:], in_=wpk[:])
            bt = constp.tile([HID, 2], F32, name="bt")
            nc.scalar.dma_start(out=bt[:], in_=bpk[:])
            xs = constp.tile([128, NC_SH], F16, name="xs")
            nss = constp.tile([HID, NC_SH], F16, name="nss")
            for i in range(NC_SH // CH):
                sl = slice(i * CH, (i + 1) * CH)
                nc.sync.dma_start(out=xs[:, sl], in_=xt[:, sl])
                nc.scalar.dma_start(out=nss[:, sl], in_=nst[:, sl])
            os = constp.tile([HID, NC_SH], F32, name="osr")
            for k in range(NC_SH // CH):
                cs = slice(k * CH, (k + 1) * CH)
                ps1 = ps1p.tile([HID, CH], F32, name="ps1", tag="ps1")
                for j in range(CH // MM):
                    ms = slice(k * CH + j * MM, k * CH + (j + 1) * MM)
                    pj = ps1[:, j * MM:(j + 1) * MM]
                    nc.tensor.matmul(pj, wt[:, 0:HID], xs[:, ms],
                                     start=True, stop=False)
                    nc.tensor.matmul(pj, wt[0:HID, HID:128], nss[:, ms],
                                     start=False, stop=True)
                r1 = workp.tile([HID, CH], F16, name="r1", tag="r1")
                nc.scalar.activation(r1[:], ps1[:], AF.Relu, bias=bt[:, 0:1])
                ps2 = ps2p.tile([HID, CH], F32, name="ps2", tag="ps2")
                for j in range(CH // MM):
                    nc.tensor.matmul(ps2[:, j * MM:(j + 1) * MM],
                                     wt[0:HID, 128:192],
                                     r1[:, j * MM:(j + 1) * MM],
                                     start=True, stop=True)
                nc.vector.tensor_scalar(os[:, cs], ps2[:], bt[:, 1:2], None,
                                        ALU.add)
                if k % 2 == 1 or k == NC_SH // CH - 1:
                    o0 = (k // 2) * 2 * CH
                    nc.scalar.dma_start(out=out[:, o0:(k + 1) * CH],
                                        in_=os[:, o0:(k + 1) * CH])
    nc.finalize()
    return nc


_CACHE = {}


def _get(name, builder):
    if name not in _CACHE:
        _CACHE[name] = builder()
    return _CACHE[name]


def _deinterleave(o):
    """[128, NCORES*EC2] chunk-pair interleaved -> [64, NCORES*EC] m^T."""
    mt = np.empty((HID, NCORES * EC), o.dtype)
    for c in range(NCORES):
        oc = o[:, c * EC2:(c + 1) * EC2]
        for s0, ssz in _slabs(EC):
            blk = oc[:, s0 // 2:(s0 + ssz) // 2]
            nq = ssz // (2 * CH)
            top = blk[:HID].reshape(HID, nq, CH)
            bot = blk[HID:].reshape(HID, nq, CH)
            mt[:, c * EC + s0:c * EC + s0 + ssz] = np.stack(
                [top, bot], axis=2).reshape(HID, ssz)
    return mt


def _interleave(mt):
    """[64, NCORES*EC] -> [128, NCORES*EC2] chunk-pair interleaved."""
    o = np.empty((2 * HID, NCORES * EC2), mt.dtype)
    for c in range(NCORES):
        for s0, ssz in _slabs(EC):
            nq = ssz // (2 * CH)
            blk = mt[:, c * EC + s0:c * EC + s0 + ssz].reshape(HID, nq, 2, CH)
            o[:HID, c * EC2 + s0 // 2:c * EC2 + (s0 + ssz) // 2] = \
                blk[:, :, 0].reshape(HID, nq * CH)
            o[HID:, c * EC2 + s0 // 2:c * EC2 + (s0 + ssz) // 2] = \
                blk[:, :, 1].reshape(HID, nq * CH)
    return o


def _run_spmd(nc, per_core, shared, trace=False):
    """per_core: dict name -> full array sliced [.., c*W:(c+1)*W] along axis 1;
       shared: dict name -> replicated array."""
    in_maps = []
    for c in range(NCORES):
        m = {}
        for k, v in per_core.items():
            w = v.shape[1] // NCORES
            m[k] = np.ascontiguousarray(v[:, c * w:(c + 1) * w])
        m.update(shared)
        in_maps.append(m)
    res = run_bass_kernel_spmd(nc, in_maps, list(range(NCORES)), trace=trace)
    outs = np.concatenate([res.results[c]["out"] for c in range(NCORES)], axis=1)
    return outs, res


def kernel(**inputs):
    x = np.asarray(inputs["x"], np.float32)
    edge_index = np.asarray(inputs["edge_index"])
    ea = np.asarray(inputs["edge_attr"], np.float32)
    W0, b0, W1, b1, W2, b2, W3, b3, W4, b4 = (
        np.asarray(inputs[k], np.float32) for k in
        ["W0", "b0", "W1", "b1", "W2", "b2", "W3", "b3", "W4", "b4"])
    src = edge_index[0].astype(np.int64)
    tgt = edge_index[1].astype(np.int64)
    E = src.shape[0]
    W1t, W1b = W1[:HID], W1[HID:]

    # reverse-pair structure (exactly the reference's construction)
    key = src * N_NODES + tgt
    order = np.argsort(key, kind="stable")
    key_sorted = key[order]
    rev_key = tgt * N_NODES + src
    lo = np.searchsorted(key_sorted, rev_key, side="left")
    hi = np.searchsorted(key_sorted, rev_key, side="right")
    special = np.nonzero(hi > lo)[0]

    # tgt-sorted order for fast host segment sums
    torder = np.argsort(tgt, kind="stable")
    tsorted = tgt[torder]
    seg_starts = np.searchsorted(tsorted, np.arange(N_NODES), side="left")
    reduce_idx = np.minimum(seg_starts, E - 1)
    empty = seg_starts == np.append(seg_starts[1:], E)

    def segsum_T(hT):
        # hT [64, E] -> ns [N, 64]
        s = np.add.reduceat(hT[:, torder], reduce_idx, axis=1)
        s[:, empty] = 0.0
        return np.ascontiguousarray(s.T)

    EPAD = NCORES * EC

    # ---- h0 on device ----
    nc_first = _get("first", _build_first)
    ea_t = np.zeros((EDGE_F, EPAD), np.float16)
    ea_t[:, :E] = ea.T
    h0t, _ = _run_spmd(nc_first, {"ea_t": ea_t},
                       {"w0": W0.astype(np.float16),
                        "b0d": np.concatenate([b0, b0]).reshape(
                            2 * HID, 1).astype(np.float32)})
    hT = _deinterleave(h0t)[:, :E].astype(np.float32)

    # ---- steps ----
    nc_step = _get("step", _build_step)
    wpk = np.zeros((2 * HID, 256), np.float16)
    wpk[:HID, :HID] = W1t.astype(np.float16)
    wpk[HID:, :HID] = np.eye(HID, dtype=np.float16)
    wpk[:HID, 2 * HID:3 * HID] = W2.astype(np.float16)
    wpk[HID:, 3 * HID:] = W2.astype(np.float16)
    b2d = np.concatenate([b2, b2]).reshape(2 * HID, 1).astype(np.float32)
    for _ in range(STEPS):
        ns = segsum_T(hT)
        t = ns @ W1b + b1
        agg1 = t[src]
        for e in special:
            rev = hT[:, order[lo[e]:hi[e]]].sum(axis=1)
            agg1[e] = (ns[src[e]] - rev) @ W1b + b1
        z = np.zeros((2 * HID, EPAD), np.float16)
        z[:HID, :E] = hT
        z[HID:, :E] = agg1.T
        mt, _ = _run_spmd(nc_step, {"z": z}, {"wpk": wpk, "b2d": b2d})
        hT += _deinterleave(mt)[:, :E].astype(np.float32)

    # ---- readout on device ----
    nc_ro = _get("readout", _build_readout)
    ns = segsum_T(hT)
    NSH = N_NODES // NCORES
    NPAD = NCORES * NC_SH
    xt = np.zeros((128, NPAD), np.float16)
    nst = np.zeros((HID, NPAD), np.float16)
    for c in range(NCORES):
        xt[:, c * NC_SH:c * NC_SH + NSH] = x[c * NSH:(c + 1) * NSH].T
        nst[:, c * NC_SH:c * NC_SH + NSH] = ns[c * NSH:(c + 1) * NSH].T
    wpk_ro = np.zeros((128, 192), np.float16)
    wpk_ro[:, :HID] = W3[:128].astype(np.float16)
    wpk_ro[:HID, HID:128] = W3[128:].astype(np.float16)
    wpk_ro[:HID, 128:] = W4.astype(np.float16)
    bpk = np.stack([b3, b4], axis=1).astype(np.float32)
    ot, _ = _run_spmd(nc_ro, {"xt": xt, "nst": nst},
                      {"wpk": wpk_ro, "bpk": bpk})
    outs = [ot[:, c * NC_SH:c * NC_SH + NSH].T for c in range(NCORES)]
    return np.ascontiguousarray(np.concatenate(outs, axis=0), dtype=np.float32)


# revision 30
# speedup vs baseline: 2.1593x; 1.0027x over previous
# Trainium2 Bass kernel for nn_DirectedMessagePassing (chemprop-style DMPNN).
#
# Device executes all dense compute (per-edge MLPs and readout MLP) as
# Bass/Tile kernels SPMD across the 8 NeuronCores, edges/nodes sharded 1/8
# per core.  Index-only segment-sum / gather plumbing between the dense
# stages runs on the host.
#
# All device I/O is fp16 in transposed [feat, elem] layout, batched into
# 4096-elem DMA slabs (8KB/partition; HWDGE fixed overhead ~650ns per DMA
# instruction would otherwise dominate).  Matmuls run in fp16 at the
# 512-col moving-operand max, two matmuls per [64, 1024] PSUM tile so the
# activation / bias ops run at 1024 cols per instruction.  Biases are folded
# into activation/vector ops.
#
#   h0 = relu(ea @ W0 + b0)                          (device, edge-sharded)
#   per step: ns = segment_sum(h by tgt); t = ns @ W1b + b1   (host)
#             agg1[e] = t[src[e]] - rev corrections          (host gather)
#             m = relu(relu(h@W1t + agg1) @ W2 + b2)          (device)
#             h += m                                          (host add)
#   out = relu([x, segsum(h)] @ W3 + b3) @ W4 + b4    (device, node-sharded)
import numpy as np

import concourse.bacc as bacc
import concourse.mybir as mybir
import concourse.tile as tile
from concourse.bass_utils import run_bass_kernel_spmd

N_NODES = 40000
N_EDGES = 400000
EDGE_F = 16
HID = 64
STEPS = 3
NCORES = 8

F32 = mybir.dt.float32
F16 = mybir.dt.float16
AF = mybir.ActivationFunctionType
ALU = mybir.AluOpType

MM = 512            # free-dim cols per matmul (fp16 moving-operand max)
CH = 1024           # cols per PSUM half-tile
SLAB = 4096         # elems per DMA instruction (8KB/partition in fp16)
EC = 51200          # padded edges per core (12*4096 + 2048; even chunks/slab)
EC2 = EC // 2       # interleaved-output width
NC_SH = 5120        # padded nodes per core


def _slabs(total):
    s = 0
    while s < total:
        sz = min(SLAB, total - s)
        yield s, sz
        s += sz


def _build_first():
    """in: ea_t [16, EC] f16 -> out [128, EC2] f16: chunk-pair interleaved
    relu(ea@W0+b0)^T — even 1024-col chunk on partitions 0:64, odd on 64:128.
    b0d is b0 stacked twice: [128, 1] f32."""
    nc = bacc.Bacc(trn_type="TRN2", num_devices=NCORES)
    ea_t = nc.dram_tensor("ea_t", [EDGE_F, EC], F16, kind="ExternalInput")
    w0 = nc.dram_tensor("w0", [EDGE_F, HID], F16, kind="ExternalInput")
    b0d = nc.dram_tensor("b0d", [2 * HID, 1], F32, kind="ExternalInput")
    out = nc.dram_tensor("out", [2 * HID, EC2], F16, kind="ExternalOutput")
    with tile.TileContext(nc) as tc:
        with (
            tc.tile_pool(name="const", bufs=1) as constp,
            tc.tile_pool(name="work", bufs=4) as workp,
            tc.tile_pool(name="ps", bufs=4, space="PSUM") as psp,
        ):
            w0t = constp.tile([EDGE_F, HID], F16, name="w0t")
            nc.sync.dma_start(out=w0t[:], in_=w0[:])
            b0t = constp.tile([2 * HID, 1], F32, name="b0t")
            nc.scalar.dma_start(out=b0t[:], in_=b0d[:])
            for s0, ssz in _slabs(EC):
                es = workp.tile([EDGE_F, SLAB], F16, name="es", tag="es")
                nc.sync.dma_start(out=es[:, :ssz], in_=ea_t[:, s0:s0 + ssz])
                os = workp.tile([2 * HID, SLAB // 2], F16, name="os", tag="os")
                for p in range(ssz // (2 * CH)):    # chunk pair -> 128 parts
                    cs = slice(p * CH, (p + 1) * CH)
                    ps = psp.tile([2 * HID, CH], F32, name="ps", tag="ps")
                    for half in range(2):
                        pr = slice(half * HID, half * HID + HID)
                        c0 = (2 * p + half) * CH
                        for j in range(CH // MM):
                            nc.tensor.matmul(
                                ps[pr, j * MM:(j + 1) * MM], w0t[:],
                                es[:, c0 + j * MM:c0 + (j + 1) * MM],
                                start=True, stop=True)
                    nc.scalar.activation(os[:, cs], ps[:], AF.Relu, bias=b0t[:])
                nc.scalar.dma_start(out=out[:, s0 // 2:(s0 + ssz) // 2],
                                    in_=os[:, :ssz // 2])
    nc.finalize()
    return nc


def _build_step():
    """in: z [128, EC] f16 (rows 0:64 h^T, 64:128 agg1^T incl. W1b+b1)
       -> out [128, EC2] f16: chunk-pair interleaved
       relu(relu(h@W1t + agg1) @ W2 + b2)^T
       wpk [128, 256] f16: cols 0:64 = [W1t; I], cols 128:256 =
       block-diag(W2, W2) so layer2 runs as one K=128/M=128 matmul per
       512-col block.  b2d [128, 1] f32 = b2 stacked twice."""
    nc = bacc.Bacc(trn_type="TRN2", num_devices=NCORES)
    z = nc.dram_tensor("z", [2 * HID, EC], F16, kind="ExternalInput")
    wpk = nc.dram_tensor("wpk", [2 * HID, 256], F16, kind="ExternalInput")
    b2d = nc.dram_tensor("b2d", [2 * HID, 1], F32, kind="ExternalInput")
    out = nc.dram_tensor("out", [2 * HID, EC2], F16, kind="ExternalOutput")
    with tile.TileContext(nc) as tc:
        with (
            tc.tile_pool(name="const", bufs=1) as constp,
            tc.tile_pool(name="work", bufs=4) as workp,
            tc.tile_pool(name="ps1", bufs=2, space="PSUM") as ps1p,
            tc.tile_pool(name="ps2", bufs=2, space="PSUM") as ps2p,
        ):
            wt = constp.tile([2 * HID, 256], F16, name="wt")
            nc.sync.dma_start(out=wt[:], in_=wpk[:])
            b2t = constp.tile([2 * HID, 1], F32, name="b2t")
            nc.scalar.dma_start(out=b2t[:], in_=b2d[:])
            for s0, ssz in _slabs(EC):
                zs = workp.tile([2 * HID, SLAB], F16, name="zs", tag="zs")
                nc.sync.dma_start(out=zs[:, :ssz], in_=z[:, s0:s0 + ssz])
                os = workp.tile([2 * HID, SLAB // 2], F16, name="os", tag="os")
                for p in range(ssz // (2 * CH)):    # chunk pair -> 128 parts
                    cs = slice(p * CH, (p + 1) * CH)
                    ps1 = ps1p.tile([2 * HID, CH], F32, name="ps1", tag="ps1")
                    for half in range(2):
                        pr = slice(half * HID, half * HID + HID)
                        c0 = (2 * p + half) * CH
                        for j in range(CH // MM):
                            nc.tensor.matmul(
                                ps1[pr, j * MM:(j + 1) * MM], wt[:, 0:HID],
                                zs[:, c0 + j * MM:c0 + (j + 1) * MM],
                                start=True, stop=True)
                    m1 = workp.tile([2 * HID, CH], F16, name="m1", tag="m1")
                    nc.scalar.activation(m1[:], ps1[:], AF.Relu)
                    ps2 = ps2p.tile([2 * HID, CH], F32, name="ps2", tag="ps2")
                    for j in range(CH // MM):
                        nc.tensor.matmul(
                            ps2[:, j * MM:(j + 1) * MM], wt[:, 128:256],
                            m1[:, j * MM:(j + 1) * MM],
                            start=True, stop=True)
                    nc.vector.tensor_scalar(os[:, cs], ps2[:], b2t[:], 0.0,
                                            ALU.add, ALU.max)
                nc.scalar.dma_start(out=out[:, s0 // 2:(s0 + ssz) // 2],
                                    in_=os[:, :ssz // 2])
    nc.finalize()
    return nc


def _build_readout():
    """in: xt [128, NC_SH] f16, nst [64, NC_SH] f16
       -> ot [64, NC_SH] f32 = (relu([x,ns]@W3+b3) @ W4 + b4)^T
       wpk [128, 192] f16: cols 0:64 = W3x, rows 0:64 cols 64:128 = W3m,
       rows 0:64 cols 128:192 = W4.  bpk [64, 2] f32: col 0 = b3, col 1 = b4."""
    nc = bacc.Bacc(trn_type="TRN2", num_devices=NCORES)
    xt = nc.dram_tensor("xt", [128, NC_SH], F16, kind="ExternalInput")
    nst = nc.dram_tensor("nst", [HID, NC_SH], F16, kind="ExternalInput")
    wpk = nc.dram_tensor("wpk", [128, 192], F16, kind="ExternalInput")
    bpk = nc.dram_tensor("bpk", [HID, 2], F32, kind="ExternalInput")
    out = nc.dram_tensor("out", [HID, NC_SH], F32, kind="ExternalOutput")
    with tile.TileContext(nc) as tc:
        with (
            tc.tile_pool(name="const", bufs=1) as constp,
            tc.tile_pool(name="work", bufs=3) as workp,
            tc.tile_pool(name="ps1", bufs=2, space="PSUM") as ps1p,
            tc.tile_pool(name="ps2", bufs=2, space="PSUM") as ps2p,
        ):
            wt = constp.tile([128, 192], F16, name="wt")
            nc.sync.dma_start(out=wt[:], in_=wpk[:])
            bt = constp.tile([HID, 2], F32, name="bt")
            nc.scalar.dma_start(out=bt[:], in_=bpk[:])
            xs = constp.tile([128, NC_SH], F16, name="xs")
            nss = constp.tile([HID, NC_SH], F16, name="nss")
            for i in range(NC_SH // CH):
                sl = slice(i * CH, (i + 1) * CH)
                nc.sync.dma_start(out=xs[:, sl], in_=xt[:, sl])
                nc.scalar.dma_start(out=nss[:, sl], in_=nst[:, sl])
            os = constp.tile([HID, NC_SH], F32, name="osr")
            for k in range(NC_SH // CH):
                cs = slice(k * CH, (k + 1) * CH)
                ps1 = ps1p.tile([HID, CH], F32, name="ps1", tag="ps1")
                for j in range(CH // MM):
                    ms = slice(k * CH + j * MM, k * CH + (j + 1) * MM)
                    pj = ps1[:, j * MM:(j + 1) * MM]
                    nc.tensor.matmul(pj, wt[:, 0:HID], xs[:, ms],
                                     start=True, stop=False)
                    nc.tensor.matmul(pj, wt[0:HID, HID:128], nss[:, ms],
                                     start=False, stop=True)
                r1 = workp.tile([HID, CH], F16, name="r1", tag="r1")
                nc.scalar.activation(r1[:], ps1[:], AF.Relu, bias=bt[:, 0:1])
                ps2 = ps2p.tile([HID, CH], F32, name="ps2", tag="ps2")
                for j in range(CH // MM):
                    nc.tensor.matmul(ps2[:, j * MM:(j + 1) * MM],
                                     wt[0:HID, 128:192],
                                     r1[:, j * MM:(j + 1) * MM],
                                     start=True, stop=True)
                nc.vector.tensor_scalar(os[:, cs], ps2[:], bt[:, 1:2], None,
                                        ALU.add)
                if k % 2 == 1 or k == NC_SH // CH - 1:
                    o0 = (k // 2) * 2 * CH
                    nc.scalar.dma_start(out=out[:, o0:(k + 1) * CH],
                                        in_=os[:, o0:(k + 1) * CH])
    nc.finalize()
    return nc


_CACHE = {}


def _get(name, builder):
    if name not in _CACHE:
        _CACHE[name] = builder()
    return _CACHE[name]


def _deinterleave(o):
    """[128, NCORES*EC2] chunk-pair interleaved -> [64, NCORES*EC] m^T."""
    mt = np.empty((HID, NCORES * EC), o.dtype)
    for c in range(NCORES):
        oc = o[:, c * EC2:(c + 1) * EC2]
        for s0, ssz in _slabs(EC):
            blk = oc[:, s0 // 2:(s0 + ssz) // 2]
            nq = ssz // (2 * CH)
            top = blk[:HID].reshape(HID, nq, CH)
            bot = blk[HID:].reshape(HID, nq, CH)
            mt[:, c * EC + s0:c * EC + s0 + ssz] = np.stack(
                [top, bot], axis=2).reshape(HID, ssz)
    return mt


def _interleave(mt):
    """[64, NCORES*EC] -> [128, NCORES*EC2] chunk-pair interleaved."""
    o = np.empty((2 * HID, NCORES * EC2), mt.dtype)
    for c in range(NCORES):
        for s0, ssz in _slabs(EC):
            nq = ssz // (2 * CH)
            blk = mt[:, c * EC + s0:c * EC + s0 + ssz].reshape(HID, nq, 2, CH)
            o[:HID, c * EC2 + s0 // 2:c * EC2 + (s0 + ssz) // 2] = \
                blk[:, :, 0].reshape(HID, nq * CH)
            o[HID:, c * EC2 + s0 // 2:c * EC2 + (s0 + ssz) // 2] = \
                blk[:, :, 1].reshape(HID, nq * CH)
    return o


def _run_spmd(nc, per_core, shared, trace=False):
    """per_core: dict name -> full array sliced [.., c*W:(c+1)*W] along axis 1;
       shared: dict name -> replicated array."""
    in_maps = []
    for c in range(NCORES):
        m = {}
        for k, v in per_core.items():
            w = v.shape[1] // NCORES
            m[k] = np.ascontiguousarray(v[:, c * w:(c + 1) * w])
        m.update(shared)
        in_maps.append(m)
    res = run_bass_kernel_spmd(nc, in_maps, list(range(NCORES)), trace=trace)
    outs = np.concatenate([res.results[c]["out"] for c in range(NCORES)], axis=1)
    return outs, res


def kernel(**inputs):
    x = np.asarray(inputs["x"], np.float32)
    edge_index = np.asarray(inputs["edge_index"])
    ea = np.asarray(inputs["edge_attr"], np.float32)
    W0, b0, W1, b1, W2, b2, W3, b3, W4, b4 = (
        np.asarray(inputs[k], np.float32) for k in
        ["W0", "b0", "W1", "b1", "W2", "b2", "W3", "b3", "W4", "b4"])
    src = edge_index[0].astype(np.int64)
    tgt = edge_index[1].astype(np.int64)
    E = src.shape[0]
    W1t, W1b = W1[:HID], W1[HID:]

    # reverse-pair structure (exactly the reference's construction)
    key = src * N_NODES + tgt
    order = np.argsort(key, kind="stable")
    key_sorted = key[order]
    rev_key = tgt * N_NODES + src
    lo = np.searchsorted(key_sorted, rev_key, side="left")
    hi = np.searchsorted(key_sorted, rev_key, side="right")
    special = np.nonzero(hi > lo)[0]

    # tgt-sorted order for fast host segment sums
    torder = np.argsort(tgt, kind="stable")
    tsorted = tgt[torder]
    seg_starts = np.searchsorted(tsorted, np.arange(N_NODES), side="left")
    reduce_idx = np.minimum(seg_starts, E - 1)
    empty = seg_starts == np.append(seg_starts[1:], E)

    def segsum_T(hT):
        # hT [64, E] -> ns [N, 64]
        s = np.add.reduceat(hT[:, torder], reduce_idx, axis=1)
        s[:, empty] = 0.0
        return np.ascontiguousarray(s.T)

    EPAD = NCORES * EC

    # ---- h0 on device ----
    nc_first = _get("first", _build_first)
    ea_t = np.zeros((EDGE_F, EPAD), np.float16)
    ea_t[:, :E] = ea.T
    h0t, _ = _run_spmd(nc_first, {"ea_t": ea_t},
                       {"w0": W0.astype(np.float16),
                        "b0d": np.concatenate([b0, b0]).reshape(
                            2 * HID, 1).astype(np.float32)})
    hT = _deinterleave(h0t)[:, :E].astype(np.float32)

    # ---- steps ----
    nc_step = _get("step", _build_step)
    wpk = np.zeros((2 * HID, 256), np.float16)
    wpk[:HID, :HID] = W1t.astype(np.float16)
    wpk[HID:, :HID] = np.eye(HID, dtype=np.float16)
    wpk[:HID, 2 * HID:3 * HID] = W2.astype(np.float16)
    wpk[HID:, 3 * HID:] = W2.astype(np.float16)
    b2d = np.concatenate([b2, b2]).reshape(2 * HID, 1).astype(np.float32)
    for _ in range(STEPS):
        ns = segsum_T(hT)
        t = ns @ W1b + b1
        agg1 = t[src]
        for e in special:
            rev = hT[:, order[lo[e]:hi[e]]].sum(axis=1)
            agg1[e] = (ns[src[e]] - rev) @ W1b + b1
        z = np.zeros((2 * HID, EPAD), np.float16)
        z[:HID, :E] = hT
        z[HID:, :E] = agg1.T
        mt, _ = _run_spmd(nc_step, {"z": z}, {"wpk": wpk, "b2d": b2d})
        hT += _deinterleave(mt)[:, :E].astype(np.float32)

    # ---- readout on device ----
    nc_ro = _get("readout", _build_readout)
    ns = segsum_T(hT)
    NSH = N_NODES // NCORES
    NPAD = NCORES * NC_SH
    xt = np.zeros((128, NPAD), np.float16)
    nst = np.zeros((HID, NPAD), np.float16)
    for c in range(NCORES):
        xt[:, c * NC_SH:c * NC_SH + NSH] = x[c * NSH:(c + 1) * NSH].T
        nst[:, c * NC_SH:c * NC_SH + NSH] = ns[c * NSH:(c + 1) * NSH].T
    wpk_ro = np.zeros((128, 192), np.float16)
    wpk_ro[:, :HID] = W3[:128].astype(np.float16)
    wpk_ro[:HID, HID:128] = W3[128:].astype(np.float16)
    wpk_ro[:HID, 128:] = W4.astype(np.float16)
    bpk = np.stack([b3, b4], axis=1).astype(np.float32)
    ot, _ = _run_spmd(nc_ro, {"xt": xt, "nst": nst},
                      {"wpk": wpk_ro, "bpk": bpk})
    outs = [ot[:, c * NC_SH:c * NC_SH + NSH].T for c in range(NCORES)]
    return np.ascontiguousarray(np.concatenate(outs, axis=0), dtype=np.float32)
